# revision 30
# baseline (speedup 1.0000x reference)
"""BarrierNet Trainium2 kernel.

Data-parallel over 8 NeuronCores: batch 8192 -> 1024 samples/core.

Per core:
  * MLP (x @ W1 -> relu -> 2 branches -> heads) on the TensorEngine in
    f32r (1 cycle/row at >=256-col moving), bias+relu/sigmoid fused into
    ScalarEngine activations reading PSUM.  Heads land as 2x [2, B]
    (c-on-partition) and are transposed to sample-major [P, 4, BC] with
    PE identity-matmul transposes into one PSUM tile + a single DVE copy
    (replaces a ~12us DRAM roundtrip).
  * QP: the reference's 300 projected-gradient-ascent dual iterations are
    reproduced exactly-enough by 1 closed-form iteration
    (z1 = -alpha*q) + 3 composed "jump" steps [51, 96, 152]:
      - A = G G^T has rank <= 2 through G ([m,2]); T un-clipped steps
        compose to lam_T = lam + T*delta - Gm Theta Gm^T delta, where
        delta = -alpha*sigma.grad is one masked fine step and
        Theta = th0*I + th1*Ghat is an analytic 2x2 function of
        Ghat = Gm^T Gm evaluated from its eigenvalues e1/e2:
        psi(e) = (alpha T - phi(e))/e, phi(e) = (1-(1-alpha e)^T)/e,
        with a Taylor branch blended in where alpha*e*T < 0.1 (exact-0
        eigenvalues are the common rank<=1 case).
      - clipped coordinates must leave the linear dynamics: mask
        sigma = (lam>0)|(grad<0) (as min(-z, a*grad)<0 on the pre-clip
        state z), Gm = sigma*G, refreshed at every jump boundary; the
        boundary clips catch mid-course constraint absorptions within
        <=2x of onset.  Validated offline in strict fp32: 6.2e-3 rel err
        vs the fp32 reference (gate 2e-2); measured on-device identically.
      - (1-alpha e)^T via DVE repeated squaring: the ACT Ln/Exp route
        costs two 1.28us ACT_TABLE_LOADs per jump.
      - coefficient math packed into multi-channel tiles ([Sxx|Syy|Sxy|dd],
        [e1|e2|disc] with one fused reciprocal, 3-channel Phi) and woven
        so dependent DVE ops are separated (dependent cadence ~200ns vs
        ~84ns issue-limited).
  * Barrier/G/alpha prep and all DMAs except W22 overlap the MLP; the
    heads-dependent tail (q assembly, packed 3-channel products) is the
    only pre-jump critical path.
"""

import numpy as np

import concourse.bass as bass
import concourse.mybir as mybir
import concourse.tile as tile
from concourse.bass_utils import run_bass_kernel_spmd

f32 = mybir.dt.float32
AF = mybir.ActivationFunctionType
Alu = mybir.AluOpType
AX = mybir.AxisListType

# f32r (TF32-like) for the MLP matmuls: at >=256-col moving it runs at the
# PE's 1 cycle/row peak (same as fp16) without cast precision loss.
MLP_DT = mybir.dt.float32r

N_CORES = 8
B_TOTAL = 8192
B_CORE = B_TOTAL // N_CORES          # 1024
P = 128                              # partitions
BC = B_CORE // P                     # 8 b-chunks
M = 9                                # 8 static obstacles + opponent
PI = float(np.pi)
R2_OPP = float(np.float32(1.1) * np.float32(1.1))  # (0.5+0.5+0.1)^2 in f32

F_FINE = 1                           # init counts as iteration 1
JUMPS = [51, 96, 152]                # sum = 299 = 300 - F_FINE


def _split_multi_waits(nc, max_waits=1):
    """This walrus build only supports one sync-wait command per
    instruction.  Move excess waits onto preceding same-engine NOPs."""
    uid = [0]
    for fn in nc.m.functions:
        for blk in fn.blocks:
            insts = blk.instructions
            new = []
            for ins in insts:
                si = getattr(ins, "sync_info", None)
                waits = list(si.on_wait) if (si is not None and si.on_wait) else []
                if len(waits) > max_waits:
                    rest = waits[max_waits:]
                    for i in range(0, len(rest), max_waits):
                        uid[0] += 1
                        new.append(mybir.InstNoOp(
                            name=f"wsplit_{uid[0]}",
                            engine=ins.engine,
                            bass_nofuse=True,
                            sync_info=mybir.SyncInfo(
                                on_wait=rest[i:i + max_waits], on_update=[]),
                        ))
                    ins.sync_info = mybir.SyncInfo(
                        on_wait=waits[:max_waits],
                        on_update=list(si.on_update) if si.on_update else [])
                new.append(ins)
            blk.instructions = new


def build_kernel():
    nc = bass.Bass()

    # ---- DRAM I/O (per core) ----
    xT_d = nc.dram_tensor("xT", (8, B_CORE), f32, kind="ExternalInput")
    W1_d = nc.dram_tensor("W1", (8, 1024), f32, kind="ExternalInput")
    b1_d = nc.dram_tensor("b1", (1024,), f32, kind="ExternalInput")
    W21_d = nc.dram_tensor("W21", (1024, 512), f32, kind="ExternalInput")
    b21_d = nc.dram_tensor("b21", (512,), f32, kind="ExternalInput")
    W22_d = nc.dram_tensor("W22", (1024, 512), f32, kind="ExternalInput")
    b22_d = nc.dram_tensor("b22", (512,), f32, kind="ExternalInput")
    W31_d = nc.dram_tensor("W31", (512, 2), f32, kind="ExternalInput")
    b31_d = nc.dram_tensor("b31", (2,), f32, kind="ExternalInput")
    W32_d = nc.dram_tensor("W32", (512, 2), f32, kind="ExternalInput")
    b32_d = nc.dram_tensor("b32", (2,), f32, kind="ExternalInput")
    xsg_d = nc.dram_tensor("xsg", (P, 8, BC), f32, kind="ExternalInput")
    obsb_d = nc.dram_tensor("obsb", (P, 3, 8), f32, kind="ExternalInput")
    stdb_d = nc.dram_tensor("stdb", (P, 8), f32, kind="ExternalInput")
    meanb_d = nc.dram_tensor("meanb", (P, 8), f32, kind="ExternalInput")
    id4_d = nc.dram_tensor("id4", (4, 4), f32, kind="ExternalInput")
    y_d = nc.dram_tensor("y", (B_CORE, 2), f32, kind="ExternalOutput")

    with tile.TileContext(nc) as tc:
        with (
            tc.tile_pool(name="w", bufs=1) as wp,
            tc.tile_pool(name="act", bufs=1) as ap,
            tc.tile_pool(name="qp", bufs=1) as qp,
            tc.tile_pool(name="scr", bufs=1) as scr,
            tc.tile_pool(name="ps", bufs=6, space="PSUM") as ps,
            tc.tile_pool(name="psh", bufs=2, space="PSUM") as psh,
        ):
            # ---------------- load ----------------
            xT = wp.tile([8, B_CORE], f32)
            W1 = wp.tile([8, 1024], f32)
            b1 = wp.tile([P, 8], f32)          # b1[p, mo] = b1_d[mo*128+p]
            W21 = wp.tile([P, 8, 512], f32)    # [p, k, n] = W21_d[k*128+p, n]
            W22 = wp.tile([P, 8, 512], f32)
            b2 = wp.tile([P, 2, 4], f32)       # [p, j, mo] = b2j_d[mo*128+p]
            W31 = wp.tile([P, 4, 2], f32)      # [p, kk, c] = W31_d[kk*128+p, c]
            W32 = wp.tile([P, 4, 2], f32)
            b3 = wp.tile([2, 2], f32)          # [c, j]: b31 | b32
            id4 = wp.tile([4, 4], f32)
            obsb = wp.tile([P, 3, 8], f32)
            stdb = wp.tile([P, 8], f32)
            meanb = wp.tile([P, 8], f32)
            xs = wp.tile([P, 8, BC], f32)      # [p, f, b] = x[b*128+p, f]

            nc.sync.dma_start(xT[:], xT_d[:])
            nc.sync.dma_start(W1[:], W1_d[:])
            nc.sync.dma_start(b1[:], b1_d.rearrange("(mo p) -> p mo", p=P))
            # W21 first (L2's k-accumulation needs all 8 chunks), then the
            # small prep tensors, then W22 (only needed at L2's midpoint)
            for k in range(8):
                nc.sync.dma_start(W21[:, k, :],
                                  W21_d.rearrange("(k p) n -> p k n", p=P)[:, k, :])
            nc.sync.dma_start(xs[:], xsg_d[:])
            nc.sync.dma_start(obsb[:], obsb_d[:])
            nc.sync.dma_start(stdb[:], stdb_d[:])
            nc.sync.dma_start(meanb[:], meanb_d[:])
            nc.sync.dma_start(b2[:, 0, :], b21_d.rearrange("(mo p) -> p mo", p=P))
            nc.sync.dma_start(b2[:, 1, :], b22_d.rearrange("(mo p) -> p mo", p=P))
            nc.sync.dma_start(W31[:], W31_d.rearrange("(kk p) c -> p kk c", p=P))
            nc.sync.dma_start(W32[:], W32_d.rearrange("(kk p) c -> p kk c", p=P))
            nc.sync.dma_start(b3[:, 0], b31_d[:].unsqueeze(0).transpose([1, 0]))
            nc.sync.dma_start(b3[:, 1], b32_d[:].unsqueeze(0).transpose([1, 0]))
            nc.sync.dma_start(id4[:], id4_d[:])
            for k in range(8):
                nc.sync.dma_start(W22[:, k, :],
                                  W22_d.rearrange("(k p) n -> p k n", p=P)[:, k, :])

            # ---------------- MLP operand casts ----------------
            W1r = wp.tile([8, 1024], MLP_DT, name="W1r")
            xTr = wp.tile([8, B_CORE], MLP_DT, name="xTr")
            W21r = wp.tile([P, 8, 512], MLP_DT, name="W21r")
            W22r = wp.tile([P, 8, 512], MLP_DT, name="W22r")
            W31r = wp.tile([P, 4, 2], MLP_DT, name="W31r")
            W32r = wp.tile([P, 4, 2], MLP_DT, name="W32r")
            # halves so the first L1 matmul starts after half the cast work
            nc.vector.tensor_copy(xTr[:, 0:512], xT[:, 0:512])
            nc.vector.tensor_copy(W1r[:, 0:512], W1[:, 0:512])
            nc.vector.tensor_copy(xTr[:, 512:1024], xT[:, 512:1024])
            nc.vector.tensor_copy(W1r[:, 512:1024], W1[:, 512:1024])
            for k in range(8):
                nc.vector.tensor_copy(W21r[:, k, :], W21[:, k, :])
                nc.vector.tensor_copy(W22r[:, k, :], W22[:, k, :])
            nc.vector.tensor_copy(W31r[:], W31[:])
            nc.vector.tensor_copy(W32r[:], W32[:])

            # L1 evacuation stays on ACT only: routing half to the DVE was
            # measured to push the prep chain (same DVE queue) 3.5us later,
            # delaying the loop start.
            NH = 512  # moving free dim per matmul
            h1T = ap.tile([P, 8, B_CORE], MLP_DT)     # [p, mo, n] : h1^T
            for mo in range(8):
                for hf in range(B_CORE // NH):
                    pt = ps.tile([P, NH], f32, name="ps_mm")
                    nc.tensor.matmul(pt[:], W1r[:, bass.ts(mo, P)],
                                     xTr[:, bass.ts(hf, NH)], start=True, stop=True)
                    nc.scalar.activation(h1T[:, mo, bass.ts(hf, NH)], pt[:],
                                         AF.Relu, bias=b1[:, mo:mo + 1])

            # L2 PSUM evacuation alternates ACT / DVE: the ACT engine's
            # (172+512)-cycle errata makes it the MLP's co-bottleneck, and
            # the DVE has slack here.  DVE path: relu(psum + bias) via
            # tensor_scalar (scalar ops run 2x for fp32).
            x2T = ap.tile([P, 2, 4, B_CORE], MLP_DT)  # [p, branch, mo, n]
            # gains branch (j=1) FIRST: its heads/transpose/pg-half complete
            # while the PE still runs the p-branch L2, so the gains-dependent
            # q-assembly strand overlaps the MLP tail.
            h31 = ap.tile([2, B_CORE], f32, name="h31")
            h32 = ap.tile([2, B_CORE], f32, name="h32")
            pg = wp.tile([P, 4, BC], f32)
            evac = 0

            for j, W2, W3, dsts in ((1, W22r, W32r, h32), (0, W21r, W31r, h31)):
                for mo in range(4):
                    for hf in range(B_CORE // NH):
                        pt = ps.tile([P, NH], f32, name="ps_mm")
                        for k in range(8):
                            nc.tensor.matmul(pt[:], W2[:, k, bass.ts(mo, P)],
                                             h1T[:, k, bass.ts(hf, NH)],
                                             start=(k == 0), stop=(k == 7))
                        dst = x2T[:, j, mo, bass.ts(hf, NH)]
                        if evac % 2 == 0:
                            nc.scalar.activation(dst, pt[:], AF.Relu,
                                                 bias=b2[:, j, mo:mo + 1])
                        else:
                            nc.vector.tensor_scalar(dst, pt[:],
                                                    b2[:, j, mo:mo + 1], 0.0,
                                                    Alu.add, Alu.max)
                        evac += 1
                # heads for this branch
                for hf in range(B_CORE // NH):
                    pt2 = psh.tile([2, NH], f32, name="ps_hd")
                    for kk in range(4):
                        nc.tensor.matmul(pt2[:], W3[:, kk, :],
                                         x2T[:, j, kk, bass.ts(hf, NH)],
                                         start=(kk == 0), stop=(kk == 3))
                    func = AF.Identity if j == 0 else AF.Sigmoid
                    nc.scalar.activation(dsts[:, bass.ts(hf, NH)],
                                         pt2[:], func, bias=b3[:, j:j + 1])
                # PE transpose to sample layout; pg ch = [p1, p2, sg1, sg2]
                ptT = ps.tile([P, 2 * BC], f32, name="ps_mm")
                for hf in range(BC):
                    nc.tensor.matmul(ptT[:, 2 * hf:2 * hf + 2],
                                     dsts[:, bass.ts(hf, P)], id4[0:2, 0:2],
                                     is_transpose=True)
                nc.vector.tensor_copy(
                    pg[:, 2 * j:2 * j + 2, :],
                    ptT[:].rearrange("p (b c) -> p c b", c=2))
            p1c, p2c = pg[:, 0, :], pg[:, 1, :]
            sg1, sg2 = pg[:, 2, :], pg[:, 3, :]

            # ---------------- barrier / QP prep ----------------
            # Everything up to (and including) alpha depends only on x/obsb
            # DMAs, so it runs on the DVE while the PE is still in the MLP.
            V = nc.vector
            gxy = qp.tile([P, 2, BC, M], f32)    # Gx | Gy
            agq = qp.tile([P, M, BC, 3], f32)    # aGx | aGy | aq
            lam = qp.tile([P, BC, M], f32)       # pre-clip dual state z
            S3 = qp.tile([P, BC, 3], f32)
            T = qp.tile([P, 2, BC, M], f32)
            Z = qp.tile([P, M, BC, 3], f32)
            Vt = qp.tile([P, BC, M], f32)
            # [m, b]-ordered views for the prep ops
            gx_mb = gxy[:, 0, :, :].transpose([0, 2, 1])   # [P, M, BC]
            gy_mb = gxy[:, 1, :, :].transpose([0, 2, 1])

            x0s = scr.tile([P, 8, BC], f32)      # un-normalized state
            t0 = scr.tile([P, 8, BC], f32)
            stdB = stdb[:].unsqueeze(2).broadcast_to([P, 8, BC])
            meanB = meanb[:].unsqueeze(2).broadcast_to([P, 8, BC])
            V.tensor_tensor(t0[:], xs[:], stdB, Alu.mult)
            V.tensor_tensor(x0s[:], t0[:], meanB, Alu.add)
            px, py, th, vv = x0s[:, 0, :], x0s[:, 1, :], x0s[:, 2, :], x0s[:, 3, :]
            oppx, oppy = x0s[:, 4, :], x0s[:, 5, :]

            # sin/cos with range wrap into [-pi, pi] (2 rounds, covers +-5pi)
            st = scr.tile([P, BC], f32)
            ct = scr.tile([P, BC], f32)
            w1t = scr.tile([P, BC], f32)
            w2t = scr.tile([P, BC], f32)
            w3t = scr.tile([P, BC], f32)

            def wrap_to(dst_ap, src_ap):
                cur = src_ap
                for _ in range(2):
                    V.tensor_scalar(w1t[:], cur, -PI, 2 * PI, Alu.is_lt, Alu.mult)
                    V.tensor_scalar(w2t[:], cur, PI, -2 * PI, Alu.is_gt, Alu.mult)
                    V.tensor_tensor(w1t[:], w1t[:], w2t[:], Alu.add)
                    V.tensor_tensor(dst_ap, w1t[:], cur, Alu.add)
                    cur = dst_ap

            wrap_to(w3t[:], th)
            nc.scalar.activation(st[:], w3t[:], AF.Sin)
            V.tensor_scalar(w3t[:], th, PI / 2, None, Alu.add)
            wrap_to(w3t[:], w3t[:])
            nc.scalar.activation(ct[:], w3t[:], AF.Sin)

            # dx, dy  [P, M, BC]
            dxP = scr.tile([P, M, BC], f32)
            dyP = scr.tile([P, M, BC], f32)
            pxB = px.unsqueeze(1).broadcast_to([P, 8, BC])
            pyB = py.unsqueeze(1).broadcast_to([P, 8, BC])
            oxB = obsb[:, 0, :].unsqueeze(2).broadcast_to([P, 8, BC])
            oyB = obsb[:, 1, :].unsqueeze(2).broadcast_to([P, 8, BC])
            V.scalar_tensor_tensor(dxP[:, 0:8, :], pxB, 1.0, oxB, Alu.mult, Alu.subtract)
            V.scalar_tensor_tensor(dyP[:, 0:8, :], pyB, 1.0, oyB, Alu.mult, Alu.subtract)
            V.tensor_tensor(dxP[:, 8, :], px, oppx, Alu.subtract)
            V.tensor_tensor(dyP[:, 8, :], py, oppy, Alu.subtract)

            # barrier = dx^2 + dy^2 - R^2
            bb3 = scr.tile([P, 3, M, BC], f32, name="bb3")
            V.memset(bb3[:, 2], 1.0)
            bar = bb3[:, 1]
            sq1 = scr.tile([P, M, BC], f32)
            V.tensor_tensor(sq1[:], dxP[:], dxP[:], Alu.mult)
            V.tensor_tensor(bar[:], dyP[:], dyP[:], Alu.mult)
            V.tensor_tensor(sq1[:], sq1[:], bar[:], Alu.add)   # dx^2+dy^2
            R2s = scr.tile([P, 8, BC], f32, name="R2s")
            orB = obsb[:, 2, :].unsqueeze(2).broadcast_to([P, 8, BC])
            V.tensor_scalar(R2s[:], orB, 0.6, None, Alu.add)
            V.tensor_tensor(R2s[:], R2s[:], R2s[:], Alu.mult)
            V.tensor_tensor(bar[:, 0:8, :], sq1[:, 0:8, :], R2s[:], Alu.subtract)
            V.tensor_scalar(bar[:, 8, :], sq1[:, 8, :], R2_OPP, None, Alu.subtract)

            # trig/velocity products
            vst = scr.tile([P, BC], f32)
            vct = scr.tile([P, BC], f32)
            nct2 = scr.tile([P, BC], f32)
            nst2 = scr.tile([P, BC], f32)
            V.scalar_tensor_tensor(vst[:], vv, 2.0, st[:], Alu.mult, Alu.mult)
            V.scalar_tensor_tensor(vct[:], vv, 2.0, ct[:], Alu.mult, Alu.mult)
            V.tensor_scalar(nct2[:], ct[:], -2.0, None, Alu.mult)
            V.tensor_scalar(nst2[:], st[:], -2.0, None, Alu.mult)
            vstB = vst[:].unsqueeze(1).broadcast_to([P, M, BC])
            vctB = vct[:].unsqueeze(1).broadcast_to([P, M, BC])
            nct2B = nct2[:].unsqueeze(1).broadcast_to([P, M, BC])
            nst2B = nst2[:].unsqueeze(1).broadcast_to([P, M, BC])

            q1 = scr.tile([P, M, BC], f32)
            q2 = scr.tile([P, M, BC], f32)
            bdot = bb3[:, 0]
            V.tensor_tensor(q1[:], dxP[:], vctB, Alu.mult)
            V.tensor_tensor(q2[:], dyP[:], vstB, Alu.mult)
            V.tensor_tensor(bdot[:], q1[:], q2[:], Alu.add)

            V.tensor_tensor(q1[:], dxP[:], vstB, Alu.mult)
            V.tensor_tensor(q2[:], dyP[:], vctB, Alu.mult)
            V.tensor_tensor(gx_mb, q1[:], q2[:], Alu.subtract)  # G1
            V.tensor_tensor(q1[:], dxP[:], nct2B, Alu.mult)
            V.tensor_tensor(q2[:], dyP[:], nst2B, Alu.mult)
            V.tensor_tensor(gy_mb, q1[:], q2[:], Alu.add)       # G2

            # alpha = 1 / (sqrt(Sxx^2 + 2*Sxy^2 + Syy^2) + 1e-6)
            # (independent of the MLP heads -- overlaps the matmuls)
            Sxx = scr.tile([P, BC], f32)
            Syy = scr.tile([P, BC], f32)
            Sxy = scr.tile([P, BC], f32)
            V.tensor_tensor(q1[:], gx_mb, gx_mb, Alu.mult)
            V.tensor_reduce(Sxx[:], q1[:].transpose([0, 2, 1]), AX.X, Alu.add)
            V.tensor_tensor(q1[:], gy_mb, gy_mb, Alu.mult)
            V.tensor_reduce(Syy[:], q1[:].transpose([0, 2, 1]), AX.X, Alu.add)
            V.tensor_tensor(q1[:], gx_mb, gy_mb, Alu.mult)
            V.tensor_reduce(Sxy[:], q1[:].transpose([0, 2, 1]), AX.X, Alu.add)
            wsum = scr.tile([P, BC], f32)
            V.tensor_tensor(wsum[:], Sxx[:], Sxx[:], Alu.mult)
            V.scalar_tensor_tensor(w1t[:], Sxy[:], 2.0, Sxy[:], Alu.mult, Alu.mult)
            V.tensor_tensor(wsum[:], wsum[:], w1t[:], Alu.add)
            V.tensor_tensor(w1t[:], Syy[:], Syy[:], Alu.mult)
            V.tensor_tensor(wsum[:], wsum[:], w1t[:], Alu.add)
            alph = scr.tile([P, BC], f32)
            Linv = scr.tile([P, BC], f32)        # ||A||_F + 1e-6  (= 1/alpha)
            nc.scalar.activation(w2t[:], wsum[:], AF.Sqrt)
            V.tensor_scalar(Linv[:], w2t[:], 1e-6, None, Alu.add)
            V.reciprocal(alph[:], Linv[:])
            alphB = alph[:].unsqueeze(1).broadcast_to([P, M, BC])
            V.tensor_tensor(agq[:, :, :, 0], gx_mb, alphB, Alu.mult)
            V.tensor_tensor(agq[:, :, :, 1], gy_mb, alphB, Alu.mult)

            V.memset(S3[:, :, 2], 1.0)

            # per-jump constants aT = alpha*T, aTa = alpha^2*T and lf2b:
            # heads-INDEPENDENT, so issued before the heads wait (the DVE
            # drains its queue in program order).
            gn3 = scr.tile([P, 3, BC], f32, name="gn3")
            V.scalar_tensor_tensor(gn3[:, 2, :], vv, 2.0, vv, Alu.mult, Alu.mult)
            NJ = len(JUMPS)
            ATH = scr.tile([P, NJ, BC], f32, name="ATH")
            ATAH = scr.tile([P, NJ, BC], f32, name="ATAH")
            for ji, Tj in enumerate(JUMPS):
                V.tensor_scalar(ATH[:, ji, :], alph[:], float(Tj), None, Alu.mult)
                V.tensor_tensor(ATAH[:, ji, :], ATH[:, ji, :], alph[:], Alu.mult)

            # --- heads-dependent tail of the prep (critical path) ---
            # h = 2v^2 + 4(s1+s2)*bdot + 16*s1*s2*barrier;  q = G.p + h
            # bb3 = [bdot | bar | ones], gn3 = [A4 | B16 | lf2b]; the two
            # strands (pr3-h and G.p) are independent -> interleaved.
            hq = scr.tile([P, M, BC], f32)
            pr3 = scr.tile([P, 3, M, BC], f32, name="pr3")
            pB2 = pg[:, 0:2, :].unsqueeze(3).broadcast_to([P, 2, BC, M])
            V.tensor_tensor(gn3[:, 0, :], sg1, sg2, Alu.add)
            V.tensor_tensor(T[:], gxy[:], pB2, Alu.mult)
            V.tensor_scalar(gn3[:, 0, :], gn3[:, 0, :], 4.0, None, Alu.mult)
            V.scalar_tensor_tensor(gn3[:, 1, :], sg1, 16.0, sg2, Alu.mult, Alu.mult)
            V.tensor_tensor(q2[:].transpose([0, 2, 1]), T[:, 0], T[:, 1], Alu.add)
            V.tensor_tensor(pr3[:], bb3[:],
                            gn3[:].unsqueeze(2).broadcast_to([P, 3, M, BC]), Alu.mult)
            V.tensor_reduce(hq[:], pr3[:].transpose([0, 2, 3, 1]), AX.X, Alu.add)
            V.tensor_tensor(hq[:], q2[:], hq[:], Alu.add)      # hq := q vector
            V.tensor_tensor(agq[:, :, :, 2], hq[:], alphB, Alu.mult)

            # iteration 1 from z=0 reduces to z_1 = -alpha*q: initialize the
            # state directly.
            V.tensor_scalar(lam[:].transpose([0, 2, 1]), agq[:, :, :, 2],
                            -1.0, None, Alu.mult)

            # ---------------- fine iterations (2..F_FINE) ----------------
            # Two sample-halves interleaved so consecutive DVE ops are
            # independent (hides the per-op pipe-drain stall).
            HB = BC // 2
            halves = [slice(0, HB), slice(HB, BC)]
            lam_b2 = [lam[:, hs, :].unsqueeze(1).broadcast_to([P, 2, HB, M])
                      for hs in halves]
            s_bM = [S3[:, hs, :].unsqueeze(1).broadcast_to([P, M, HB, 3])
                    for hs in halves]
            for it in range(F_FINE - 1):
                for i, hs in enumerate(halves):
                    V.scalar_tensor_tensor(T[:, :, hs, :], lam_b2[i], 0.0,
                                           gxy[:, :, hs, :], Alu.max, Alu.mult)
                for i, hs in enumerate(halves):
                    V.tensor_reduce(S3[:, hs, 0:2].transpose([0, 2, 1]),
                                    T[:, :, hs, :], AX.X, Alu.add)
                for i, hs in enumerate(halves):
                    V.tensor_tensor(Z[:, :, hs, :], agq[:, :, hs, :], s_bM[i], Alu.mult)
                for i, hs in enumerate(halves):
                    V.tensor_reduce(Vt[:, hs, :].transpose([0, 2, 1]),
                                    Z[:, :, hs, :], AX.X, Alu.add)
                for i, hs in enumerate(halves):
                    V.scalar_tensor_tensor(lam[:, hs, :], lam[:, hs, :], 0.0,
                                           Vt[:, hs, :], Alu.max, Alu.subtract)

            # ---------------- composed jumps ----------------
            # views
            alph2 = alph[:].unsqueeze(1).broadcast_to([P, 2, BC])
            Linv2 = Linv[:].unsqueeze(1).broadcast_to([P, 2, BC])

            TP = qp.tile([P, 2, BC, M], f32, name="TP")
            GM = qp.tile([P, 2, BC, M], f32, name="GM")
            GS = qp.tile([P, BC, M], f32, name="GS")
            GR = qp.tile([P, BC, M], f32, name="GR")
            SGm = qp.tile([P, BC, M], f32, name="SGm")
            Dlt = qp.tile([P, BC, M], f32, name="Dlt")
            GL = qp.tile([P, 2, BC], f32, name="GL")
            Sdg4 = qp.tile([P, 4, BC], f32, name="Sdg4")   # Sxx|Syy|Sxy|dd
            E3 = qp.tile([P, 3, BC], f32, name="E3")       # e1|e2|disc
            IED = qp.tile([P, 3, BC], f32, name="IED")     # 1/(e1+fl)|1/(e2+fl)|1/(disc+fl)
            SQ2 = qp.tile([P, 2, BC], f32, name="SQ2")
            F3 = qp.tile([P, 3, BC], f32, name="F3")
            GD = qp.tile([P, 2, BC], f32, name="GD")
            Se = qp.tile([P, 2, BC], f32, name="Se")
            Re = qp.tile([P, 2, BC], f32, name="Re")
            LNe = qp.tile([P, 2, BC], f32, name="LNe")
            EXe = qp.tile([P, 2, BC], f32, name="EXe")
            NUMe = qp.tile([P, 2, BC], f32, name="NUMe")
            PHI = qp.tile([P, 2, BC], f32, name="PHI")
            PSS = qp.tile([P, 2, BC], f32, name="PSS")
            PSI = qp.tile([P, 2, BC], f32, name="PSI")
            SER = qp.tile([P, 2, BC], f32, name="SER")
            WSL = qp.tile([P, 2, BC], f32, name="WSL")
            A1 = qp.tile([P, 2, BC], f32, name="A1")
            B1t = qp.tile([P, 2, BC], f32, name="B1t")
            Wv = qp.tile([P, 2, BC], f32, name="Wv")
            PWS = {k: qp.tile([P, 2, BC], f32, name=f"PW{k}")
                   for k in range(1, 8)}
            TRt = scr.tile([P, BC], f32, name="TRt")
            HTt = scr.tile([P, BC], f32, name="HTt")
            FLt = scr.tile([P, BC], f32, name="FLt")
            TH0 = scr.tile([P, BC], f32, name="TH0")
            TH1 = scr.tile([P, BC], f32, name="TH1")
            tA = scr.tile([P, BC], f32, name="tA")
            tB = scr.tile([P, BC], f32, name="tB")

            lamB = lam[:].unsqueeze(1).broadcast_to([P, 2, BC, M])
            sgB = SGm[:].unsqueeze(1).broadcast_to([P, 2, BC, M])
            dB = Dlt[:].unsqueeze(1).broadcast_to([P, 2, BC, M])
            wB = Wv[:].unsqueeze(3).broadcast_to([P, 2, BC, M])
            flB3 = FLt[:].unsqueeze(1).broadcast_to([P, 3, BC])
            th0B = TH0[:].unsqueeze(1).broadcast_to([P, 2, BC])
            th1B3 = TH1[:].unsqueeze(1).broadcast_to([P, 3, BC])
            phoB = F3[:, 2, :].unsqueeze(1).broadcast_to([P, 2, BC])

            sg_b2 = [SGm[:, hs, :].unsqueeze(1).broadcast_to([P, 2, HB, M])
                     for hs in halves]
            for ji, Tj in enumerate(JUMPS):
                Tf = float(Tj)
                ataB = ATAH[:, ji, :].unsqueeze(1).broadcast_to([P, 2, BC])
                atB = ATH[:, ji, :].unsqueeze(1).broadcast_to([P, 2, BC])
                # ---- alpha*grad via the agq 3-channel trick; the strictly
                # sequential M-chain runs as two interleaved sample-halves so
                # each dependent pair is separated by the other half's op.
                for i, hs in enumerate(halves):
                    V.scalar_tensor_tensor(T[:, :, hs, :], lam_b2[i], 0.0,
                                           gxy[:, :, hs, :], Alu.max, Alu.mult)
                for i, hs in enumerate(halves):
                    V.tensor_reduce(S3[:, hs, 0:2].transpose([0, 2, 1]),
                                    T[:, :, hs, :], AX.X, Alu.add)
                for i, hs in enumerate(halves):
                    V.tensor_tensor(Z[:, :, hs, :], agq[:, :, hs, :], s_bM[i],
                                    Alu.mult)
                for i, hs in enumerate(halves):
                    V.tensor_reduce(Vt[:, hs, :].transpose([0, 2, 1]),
                                    Z[:, :, hs, :], AX.X, Alu.add)
                # sigma = (lam>0)|(grad<0) via min(-z, a*grad) < 0
                for i, hs in enumerate(halves):
                    V.scalar_tensor_tensor(GS[:, hs, :], lam[:, hs, :], -1.0,
                                           Vt[:, hs, :], Alu.mult, Alu.min)
                for i, hs in enumerate(halves):
                    V.tensor_scalar(SGm[:, hs, :], GS[:, hs, :], 0.0, None,
                                    Alu.is_lt)
                for i, hs in enumerate(halves):
                    V.tensor_tensor(GM[:, :, hs, :], gxy[:, :, hs, :], sg_b2[i],
                                    Alu.mult)
                for i, hs in enumerate(halves):
                    V.scalar_tensor_tensor(Dlt[:, hs, :], Vt[:, hs, :], -1.0,
                                           SGm[:, hs, :], Alu.mult, Alu.mult)
                # ---- masked Ghat; disc chain first so Sqrt issues early ----
                V.tensor_tensor(TP[:], GM[:], GM[:], Alu.mult)
                V.tensor_tensor(GS[:], GM[:, 0], GM[:, 1], Alu.mult)
                V.tensor_reduce(Sdg4[:, 0:2, :], TP[:], AX.X, Alu.add)  # Sxx|Syy
                V.tensor_reduce(Sdg4[:, 2, :], GS[:], AX.X, Alu.add)    # Sxy
                V.tensor_tensor(Sdg4[:, 3, :], Sdg4[:, 0, :], Sdg4[:, 1, :],
                                Alu.subtract)                           # dd
                V.tensor_tensor(SQ2[:], Sdg4[:, 2:4, :], Sdg4[:, 2:4, :], Alu.mult)
                V.scalar_tensor_tensor(tA[:], SQ2[:, 0], 4.0, SQ2[:, 1],
                                       Alu.mult, Alu.add)               # disc^2
                nc.scalar.activation(E3[:, 2, :], tA[:], AF.Sqrt)
                # (fill Sqrt latency with independent work)
                V.tensor_tensor(TP[:], GM[:], dB, Alu.mult)            # Gm*delta
                V.tensor_tensor(TRt[:], Sdg4[:, 0, :], Sdg4[:, 1, :], Alu.add)
                V.tensor_scalar(FLt[:], TRt[:], 1e-6, 1e-12, Alu.mult, Alu.add)
                V.tensor_scalar(HTt[:], TRt[:], 0.5, None, Alu.mult)
                # ---- eigenvalues / reciprocals (packed e1|e2|disc) ----
                V.scalar_tensor_tensor(E3[:, 0, :], E3[:, 2, :], 0.5, HTt[:],
                                       Alu.mult, Alu.add)
                V.scalar_tensor_tensor(E3[:, 1, :], E3[:, 2, :], -0.5, HTt[:],
                                       Alu.mult, Alu.add)
                V.tensor_tensor(IED[:], E3[:], flB3, Alu.add)
                V.tensor_tensor(Se[:], E3[:, 0:2, :], alph2, Alu.mult)
                V.reciprocal(IED[:], IED[:])
                V.tensor_scalar(Re[:], Se[:], -1.0, 1.0, Alu.mult, Alu.add)
                # r^T by repeated squaring on the DVE (r in [0,1]): the ACT
                # Ln/Exp route costs two 1.28us ACT_TABLE_LOADs per jump.
                # Weave the series strand + gdelta reduce into the chain.
                c1s = (Tf - 1.0) / 2.0
                c2s = (Tf - 1.0) * (Tf - 2.0) / 6.0
                c3s = (Tf - 1.0) * (Tf - 2.0) * (Tf - 3.0) / 24.0
                bits = [k for k in range(Tj.bit_length()) if (Tj >> k) & 1]
                series_ops = [
                    lambda: V.tensor_scalar(SER[:], Se[:], c3s, -c2s, Alu.mult, Alu.add),
                    lambda: V.tensor_scalar(WSL[:], Se[:], Tf, 0.1, Alu.mult, Alu.is_lt),
                    lambda: V.tensor_tensor(SER[:], SER[:], Se[:], Alu.mult),
                    lambda: V.tensor_scalar(SER[:], SER[:], c1s, None, Alu.add),
                    lambda: V.tensor_tensor(PSS[:], SER[:], ataB, Alu.mult),
                    lambda: V.tensor_reduce(GD[:], TP[:], AX.X, Alu.add),
                ]
                si = 0
                PW = {0: Re}
                for k in range(1, bits[-1] + 1):
                    V.tensor_tensor(PWS[k][:], PW[k - 1][:], PW[k - 1][:], Alu.mult)
                    PW[k] = PWS[k]
                    if si < len(series_ops):
                        series_ops[si](); si += 1
                acc = PW[bits[-1]]
                for i, k in enumerate(reversed(bits[:-1])):
                    dst = EXe if i % 2 == 0 else LNe
                    V.tensor_tensor(dst[:], acc[:], PW[k][:], Alu.mult)
                    if si < len(series_ops):
                        series_ops[si](); si += 1
                    acc = dst
                while si < len(series_ops):
                    series_ops[si](); si += 1
                EXr = acc           # holds r^T
                # ---- psi -> theta ----
                V.tensor_scalar(NUMe[:], EXr[:], -1.0, 1.0, Alu.mult, Alu.add)
                V.tensor_tensor(PHI[:], NUMe[:], IED[:, 0:2, :], Alu.mult)
                V.tensor_tensor(PSI[:], atB, PHI[:], Alu.subtract)
                V.tensor_tensor(PSI[:], PSI[:], IED[:, 0:2, :], Alu.mult)
                # blend psi_series where s*T < 0.1
                V.tensor_tensor(PHI[:], PSS[:], PSI[:], Alu.subtract)
                V.tensor_tensor(PHI[:], PHI[:], WSL[:], Alu.mult)
                V.tensor_tensor(PSI[:], PSI[:], PHI[:], Alu.add)
                V.tensor_tensor(PSI[:], PSI[:], Linv2, Alu.mult)   # psi*Linv
                V.tensor_tensor(tA[:], PSI[:, 0], PSI[:, 1], Alu.subtract)
                V.tensor_tensor(TH1[:], tA[:], IED[:, 2, :], Alu.mult)
                V.tensor_tensor(tB[:], TH1[:], E3[:, 0, :], Alu.mult)
                V.tensor_tensor(F3[:], Sdg4[:, 0:3, :], th1B3, Alu.mult)
                V.tensor_tensor(TH0[:], PSI[:, 0], tB[:], Alu.subtract)
                V.tensor_tensor(F3[:, 0:2, :], F3[:, 0:2, :], th0B, Alu.add)
                V.tensor_tensor(B1t[:], GD[:], phoB, Alu.mult)     # [o*g1, o*g2]
                V.tensor_tensor(A1[:], F3[:, 0:2, :], GD[:], Alu.mult)
                V.tensor_tensor(Wv[:, 0], A1[:, 0], B1t[:, 1], Alu.add)
                V.tensor_tensor(Wv[:, 1], A1[:, 1], B1t[:, 0], Alu.add)
                # ---- apply: z' = relu(z) + T*delta - Gm w ----
                for i, hs in enumerate(halves):
                    V.tensor_tensor(TP[:, :, hs, :], GM[:, :, hs, :],
                                    Wv[:, :, hs].unsqueeze(3).broadcast_to(
                                        [P, 2, HB, M]), Alu.mult)
                for i, hs in enumerate(halves):
                    V.tensor_tensor(GS[:, hs, :], TP[:, 0, hs, :],
                                    TP[:, 1, hs, :], Alu.add)
                for i, hs in enumerate(halves):
                    V.scalar_tensor_tensor(GR[:, hs, :], Dlt[:, hs, :], Tf,
                                           GS[:, hs, :], Alu.mult, Alu.subtract)
                for i, hs in enumerate(halves):
                    V.scalar_tensor_tensor(lam[:, hs, :], lam[:, hs, :], 0.0,
                                           GR[:, hs, :], Alu.max, Alu.add)

            # ---------------- u = -p - G^T relu(lam) ----------------
            V.scalar_tensor_tensor(TP[:], lamB, 0.0, gxy[:], Alu.max, Alu.mult)
            V.tensor_reduce(GL[:], TP[:], AX.X, Alu.add)
            u12 = scr.tile([P, BC, 2], f32)
            V.scalar_tensor_tensor(u12[:, :, 0], GL[:, 0], -1.0, p1c, Alu.mult, Alu.subtract)
            V.scalar_tensor_tensor(u12[:, :, 1], GL[:, 1], -1.0, p2c, Alu.mult, Alu.subtract)
            nc.sync.dma_start(y_d.rearrange("(b p) c -> p b c", p=P), u12[:])

    nc.finalize()
    _split_multi_waits(nc)
    return nc


_CACHED = {}


def _get_kernel():
    if "nc" not in _CACHED:
        _CACHED["nc"] = build_kernel()
    return _CACHED["nc"]


def _round_tf32(a):
    """RNE to 10-bit mantissa (TF32) so f32r consumers see pre-rounded data."""
    v = np.ascontiguousarray(np.asarray(a, np.float32)).view(np.uint32)
    r = v + np.uint32(0xFFF) + ((v >> np.uint32(13)) & np.uint32(1))
    r &= np.uint32(0xFFFFE000)
    return r.view(np.float32)


def build_in_maps(inputs):
    x = np.ascontiguousarray(np.asarray(inputs["x"], dtype=np.float32))
    obstacles = np.asarray(inputs["obstacles"], dtype=np.float32)
    std = np.asarray(inputs["std"], dtype=np.float32)
    mean = np.asarray(inputs["mean"], dtype=np.float32)

    rw = _round_tf32
    shared = {
        "W1": rw(inputs["W1"]),
        "b1": np.ascontiguousarray(np.asarray(inputs["b1"], np.float32)),
        "W21": rw(inputs["W21"]),
        "b21": np.ascontiguousarray(np.asarray(inputs["b21"], np.float32)),
        "W22": rw(inputs["W22"]),
        "b22": np.ascontiguousarray(np.asarray(inputs["b22"], np.float32)),
        "W31": rw(inputs["W31"]),
        "b31": np.ascontiguousarray(np.asarray(inputs["b31"], np.float32)),
        "W32": rw(inputs["W32"]),
        "b32": np.ascontiguousarray(np.asarray(inputs["b32"], np.float32)),
        "id4": np.eye(4, dtype=np.float32),
        "obsb": np.ascontiguousarray(
            np.broadcast_to(obstacles.T[None, :, :], (P, 3, 8)).astype(np.float32)),
        "stdb": np.ascontiguousarray(np.broadcast_to(std[None, :], (P, 8))),
        "meanb": np.ascontiguousarray(np.broadcast_to(mean[None, :], (P, 8))),
    }

    in_maps = []
    for c in range(N_CORES):
        xe = x[c * B_CORE:(c + 1) * B_CORE]            # [1024, 8]
        m = dict(shared)
        m["xT"] = rw(xe.T)                             # [8, 1024] (TF32-rounded)
        # sample-layout gather for the barrier math:
        m["xsg"] = np.ascontiguousarray(
            xe.reshape(BC, P, 8).transpose(1, 2, 0))   # [p, f, b]
        in_maps.append(m)
    return in_maps


def kernel(**inputs):
    in_maps = build_in_maps(inputs)
    nc = _get_kernel()
    res = run_bass_kernel_spmd(nc, in_maps, core_ids=list(range(N_CORES)))
    out = np.concatenate([res.results[c]["y"] for c in range(N_CORES)], axis=0)
    return out.astype(np.float32)


# revision 31
# speedup vs baseline: 1.0359x; 1.0359x over previous
"""BarrierNet Trainium2 kernel.

Data-parallel over 8 NeuronCores: batch 8192 -> 1024 samples/core.

Per core:
  * MLP (x @ W1 -> relu -> 2 branches -> heads) on the TensorEngine in
    f32r (1 cycle/row at >=256-col moving), bias+relu/sigmoid fused into
    ScalarEngine activations reading PSUM.  Heads land as 2x [2, B]
    (c-on-partition) and are transposed to sample-major [P, 4, BC] with
    PE identity-matmul transposes into one PSUM tile + a single DVE copy
    (replaces a ~12us DRAM roundtrip).
  * QP: the reference's 300 projected-gradient-ascent dual iterations are
    reproduced exactly-enough by 1 closed-form iteration
    (z1 = -alpha*q) + 3 composed "jump" steps [51, 96, 152]:
      - A = G G^T has rank <= 2 through G ([m,2]); T un-clipped steps
        compose to lam_T = lam + T*delta - Gm Theta Gm^T delta, where
        delta = -alpha*sigma.grad is one masked fine step and
        Theta = th0*I + th1*Ghat is an analytic 2x2 function of
        Ghat = Gm^T Gm evaluated from its eigenvalues e1/e2:
        psi(e) = (alpha T - phi(e))/e, phi(e) = (1-(1-alpha e)^T)/e,
        with a Taylor branch blended in where alpha*e*T < 0.1 (exact-0
        eigenvalues are the common rank<=1 case).
      - clipped coordinates must leave the linear dynamics: mask
        sigma = (lam>0)|(grad<0) (as min(-z, a*grad)<0 on the pre-clip
        state z), Gm = sigma*G, refreshed at every jump boundary; the
        boundary clips catch mid-course constraint absorptions within
        <=2x of onset.  Validated offline in strict fp32: 6.2e-3 rel err
        vs the fp32 reference (gate 2e-2); measured on-device identically.
      - (1-alpha e)^T via DVE repeated squaring: the ACT Ln/Exp route
        costs two 1.28us ACT_TABLE_LOADs per jump.
      - coefficient math packed into multi-channel tiles ([Sxx|Syy|Sxy|dd],
        [e1|e2|disc] with one fused reciprocal, 3-channel Phi) and woven
        so dependent DVE ops are separated (dependent cadence ~200ns vs
        ~84ns issue-limited).
  * Barrier/G/alpha prep and all DMAs except W22 overlap the MLP; the
    heads-dependent tail (q assembly, packed 3-channel products) is the
    only pre-jump critical path.
"""

import numpy as np

import concourse.bass as bass
import concourse.mybir as mybir
import concourse.tile as tile
from concourse.bass_utils import run_bass_kernel_spmd

f32 = mybir.dt.float32
AF = mybir.ActivationFunctionType
Alu = mybir.AluOpType
AX = mybir.AxisListType

# f32r (TF32-like) for the MLP matmuls: at >=256-col moving it runs at the
# PE's 1 cycle/row peak (same as fp16) without cast precision loss.
MLP_DT = mybir.dt.float32r

N_CORES = 8
B_TOTAL = 8192
B_CORE = B_TOTAL // N_CORES          # 1024
P = 128                              # partitions
BC = B_CORE // P                     # 8 b-chunks
M = 9                                # 8 static obstacles + opponent
PI = float(np.pi)
R2_OPP = float(np.float32(1.1) * np.float32(1.1))  # (0.5+0.5+0.1)^2 in f32

F_FINE = 1                           # init counts as iteration 1
JUMPS = [51, 96, 152]                # sum = 299 = 300 - F_FINE


def _split_multi_waits(nc, max_waits=1):
    """This walrus build only supports one sync-wait command per
    instruction.  Move excess waits onto preceding same-engine NOPs."""
    uid = [0]
    for fn in nc.m.functions:
        for blk in fn.blocks:
            insts = blk.instructions
            new = []
            for ins in insts:
                si = getattr(ins, "sync_info", None)
                waits = list(si.on_wait) if (si is not None and si.on_wait) else []
                if len(waits) > max_waits:
                    rest = waits[max_waits:]
                    for i in range(0, len(rest), max_waits):
                        uid[0] += 1
                        new.append(mybir.InstNoOp(
                            name=f"wsplit_{uid[0]}",
                            engine=ins.engine,
                            bass_nofuse=True,
                            sync_info=mybir.SyncInfo(
                                on_wait=rest[i:i + max_waits], on_update=[]),
                        ))
                    ins.sync_info = mybir.SyncInfo(
                        on_wait=waits[:max_waits],
                        on_update=list(si.on_update) if si.on_update else [])
                new.append(ins)
            blk.instructions = new


def build_kernel():
    nc = bass.Bass()

    # ---- DRAM I/O (per core) ----
    xT_d = nc.dram_tensor("xT", (8, B_CORE), f32, kind="ExternalInput")
    W1_d = nc.dram_tensor("W1", (8, 1024), f32, kind="ExternalInput")
    b1_d = nc.dram_tensor("b1", (1024,), f32, kind="ExternalInput")
    W21_d = nc.dram_tensor("W21", (1024, 512), f32, kind="ExternalInput")
    b21_d = nc.dram_tensor("b21", (512,), f32, kind="ExternalInput")
    W22_d = nc.dram_tensor("W22", (1024, 512), f32, kind="ExternalInput")
    b22_d = nc.dram_tensor("b22", (512,), f32, kind="ExternalInput")
    W31_d = nc.dram_tensor("W31", (512, 2), f32, kind="ExternalInput")
    b31_d = nc.dram_tensor("b31", (2,), f32, kind="ExternalInput")
    W32_d = nc.dram_tensor("W32", (512, 2), f32, kind="ExternalInput")
    b32_d = nc.dram_tensor("b32", (2,), f32, kind="ExternalInput")
    xsg_d = nc.dram_tensor("xsg", (P, 8, BC), f32, kind="ExternalInput")
    obsb_d = nc.dram_tensor("obsb", (P, 3, 8), f32, kind="ExternalInput")
    stdb_d = nc.dram_tensor("stdb", (P, 8), f32, kind="ExternalInput")
    meanb_d = nc.dram_tensor("meanb", (P, 8), f32, kind="ExternalInput")
    id4_d = nc.dram_tensor("id4", (4, 4), f32, kind="ExternalInput")
    y_d = nc.dram_tensor("y", (B_CORE, 2), f32, kind="ExternalOutput")

    with tile.TileContext(nc) as tc:
        with (
            tc.tile_pool(name="w", bufs=1) as wp,
            tc.tile_pool(name="act", bufs=1) as ap,
            tc.tile_pool(name="qp", bufs=1) as qp,
            tc.tile_pool(name="scr", bufs=1) as scr,
            tc.tile_pool(name="ps", bufs=6, space="PSUM") as ps,
            tc.tile_pool(name="psh", bufs=2, space="PSUM") as psh,
        ):
            # ---------------- load ----------------
            xT = wp.tile([8, B_CORE], f32)
            W1 = wp.tile([8, 1024], f32)
            b1 = wp.tile([P, 8], f32)          # b1[p, mo] = b1_d[mo*128+p]
            W21 = wp.tile([P, 8, 512], f32)    # [p, k, n] = W21_d[k*128+p, n]
            W22 = wp.tile([P, 8, 512], f32)
            b2 = wp.tile([P, 2, 4], f32)       # [p, j, mo] = b2j_d[mo*128+p]
            W31 = wp.tile([P, 4, 2], f32)      # [p, kk, c] = W31_d[kk*128+p, c]
            W32 = wp.tile([P, 4, 2], f32)
            b3 = wp.tile([2, 2], f32)          # [c, j]: b31 | b32
            id4 = wp.tile([4, 4], f32)
            obsb = wp.tile([P, 3, 8], f32)
            stdb = wp.tile([P, 8], f32)
            meanb = wp.tile([P, 8], f32)
            xs = wp.tile([P, 8, BC], f32)      # [p, f, b] = x[b*128+p, f]

            nc.sync.dma_start(xT[:], xT_d[:])
            nc.sync.dma_start(W1[:], W1_d[:])
            nc.sync.dma_start(b1[:], b1_d.rearrange("(mo p) -> p mo", p=P))
            # W22 first (the gains branch j=1 runs first and needs all 8
            # chunks), then the small prep tensors, then W21 (L2 midpoint)
            for k in range(8):
                nc.sync.dma_start(W22[:, k, :],
                                  W22_d.rearrange("(k p) n -> p k n", p=P)[:, k, :])
            nc.sync.dma_start(xs[:], xsg_d[:])
            nc.sync.dma_start(obsb[:], obsb_d[:])
            nc.sync.dma_start(stdb[:], stdb_d[:])
            nc.sync.dma_start(meanb[:], meanb_d[:])
            nc.sync.dma_start(b2[:, 0, :], b21_d.rearrange("(mo p) -> p mo", p=P))
            nc.sync.dma_start(b2[:, 1, :], b22_d.rearrange("(mo p) -> p mo", p=P))
            nc.sync.dma_start(W31[:], W31_d.rearrange("(kk p) c -> p kk c", p=P))
            nc.sync.dma_start(W32[:], W32_d.rearrange("(kk p) c -> p kk c", p=P))
            nc.sync.dma_start(b3[:, 0], b31_d[:].unsqueeze(0).transpose([1, 0]))
            nc.sync.dma_start(b3[:, 1], b32_d[:].unsqueeze(0).transpose([1, 0]))
            nc.sync.dma_start(id4[:], id4_d[:])
            for k in range(8):
                nc.sync.dma_start(W21[:, k, :],
                                  W21_d.rearrange("(k p) n -> p k n", p=P)[:, k, :])

            # ---------------- MLP operand casts ----------------
            W1r = wp.tile([8, 1024], MLP_DT, name="W1r")
            xTr = wp.tile([8, B_CORE], MLP_DT, name="xTr")
            W21r = wp.tile([P, 8, 512], MLP_DT, name="W21r")
            W22r = wp.tile([P, 8, 512], MLP_DT, name="W22r")
            W31r = wp.tile([P, 4, 2], MLP_DT, name="W31r")
            W32r = wp.tile([P, 4, 2], MLP_DT, name="W32r")
            # halves so the first L1 matmul starts after half the cast work
            nc.vector.tensor_copy(xTr[:, 0:512], xT[:, 0:512])
            nc.vector.tensor_copy(W1r[:, 0:512], W1[:, 0:512])
            nc.vector.tensor_copy(xTr[:, 512:1024], xT[:, 512:1024])
            nc.vector.tensor_copy(W1r[:, 512:1024], W1[:, 512:1024])
            for k in range(8):
                nc.vector.tensor_copy(W22r[:, k, :], W22[:, k, :])
            for k in range(8):
                nc.vector.tensor_copy(W21r[:, k, :], W21[:, k, :])
            nc.vector.tensor_copy(W31r[:], W31[:])
            nc.vector.tensor_copy(W32r[:], W32[:])

            # L1 evacuation stays on ACT only: routing half to the DVE was
            # measured to push the prep chain (same DVE queue) 3.5us later,
            # delaying the loop start.
            NH = 512  # moving free dim per matmul
            h1T = ap.tile([P, 8, B_CORE], MLP_DT)     # [p, mo, n] : h1^T
            for mo in range(8):
                for hf in range(B_CORE // NH):
                    pt = ps.tile([P, NH], f32, name="ps_mm")
                    nc.tensor.matmul(pt[:], W1r[:, bass.ts(mo, P)],
                                     xTr[:, bass.ts(hf, NH)], start=True, stop=True)
                    nc.scalar.activation(h1T[:, mo, bass.ts(hf, NH)], pt[:],
                                         AF.Relu, bias=b1[:, mo:mo + 1])

            # L2 PSUM evacuation alternates ACT / DVE: the ACT engine's
            # (172+512)-cycle errata makes it the MLP's co-bottleneck, and
            # the DVE has slack here.  DVE path: relu(psum + bias) via
            # tensor_scalar (scalar ops run 2x for fp32).
            x2T = ap.tile([P, 2, 4, B_CORE], MLP_DT)  # [p, branch, mo, n]
            # gains branch (j=1) FIRST: its heads/transpose/pg-half complete
            # while the PE still runs the p-branch L2, so the gains-dependent
            # q-assembly strand overlaps the MLP tail.
            h31 = ap.tile([2, B_CORE], f32, name="h31")
            h32 = ap.tile([2, B_CORE], f32, name="h32")
            pg = wp.tile([P, 4, BC], f32)
            evac = 0

            for j, W2, W3, dsts in ((1, W22r, W32r, h32), (0, W21r, W31r, h31)):
                for mo in range(4):
                    for hf in range(B_CORE // NH):
                        pt = ps.tile([P, NH], f32, name="ps_mm")
                        for k in range(8):
                            nc.tensor.matmul(pt[:], W2[:, k, bass.ts(mo, P)],
                                             h1T[:, k, bass.ts(hf, NH)],
                                             start=(k == 0), stop=(k == 7))
                        dst = x2T[:, j, mo, bass.ts(hf, NH)]
                        if evac % 2 == 0:
                            nc.scalar.activation(dst, pt[:], AF.Relu,
                                                 bias=b2[:, j, mo:mo + 1])
                        else:
                            nc.vector.tensor_scalar(dst, pt[:],
                                                    b2[:, j, mo:mo + 1], 0.0,
                                                    Alu.add, Alu.max)
                        evac += 1
                # heads for this branch
                for hf in range(B_CORE // NH):
                    pt2 = psh.tile([2, NH], f32, name="ps_hd")
                    for kk in range(4):
                        nc.tensor.matmul(pt2[:], W3[:, kk, :],
                                         x2T[:, j, kk, bass.ts(hf, NH)],
                                         start=(kk == 0), stop=(kk == 3))
                    func = AF.Identity if j == 0 else AF.Sigmoid
                    nc.scalar.activation(dsts[:, bass.ts(hf, NH)],
                                         pt2[:], func, bias=b3[:, j:j + 1])
                # PE transpose to sample layout; pg ch = [p1, p2, sg1, sg2]
                ptT = ps.tile([P, 2 * BC], f32, name="ps_mm")
                for hf in range(BC):
                    nc.tensor.matmul(ptT[:, 2 * hf:2 * hf + 2],
                                     dsts[:, bass.ts(hf, P)], id4[0:2, 0:2],
                                     is_transpose=True)
                nc.vector.tensor_copy(
                    pg[:, 2 * j:2 * j + 2, :],
                    ptT[:].rearrange("p (b c) -> p c b", c=2))
            p1c, p2c = pg[:, 0, :], pg[:, 1, :]
            sg1, sg2 = pg[:, 2, :], pg[:, 3, :]

            # ---------------- barrier / QP prep ----------------
            # Everything up to (and including) alpha depends only on x/obsb
            # DMAs, so it runs on the DVE while the PE is still in the MLP.
            V = nc.vector
            gxy = qp.tile([P, 2, BC, M], f32)    # Gx | Gy
            agq = qp.tile([P, M, BC, 3], f32)    # aGx | aGy | aq
            lam = qp.tile([P, BC, M], f32)       # pre-clip dual state z
            S3 = qp.tile([P, BC, 3], f32)
            T = qp.tile([P, 2, BC, M], f32)
            Z = qp.tile([P, M, BC, 3], f32)
            Vt = qp.tile([P, BC, M], f32)
            # [m, b]-ordered views for the prep ops
            gx_mb = gxy[:, 0, :, :].transpose([0, 2, 1])   # [P, M, BC]
            gy_mb = gxy[:, 1, :, :].transpose([0, 2, 1])

            x0s = scr.tile([P, 8, BC], f32)      # un-normalized state
            t0 = scr.tile([P, 8, BC], f32)
            stdB = stdb[:].unsqueeze(2).broadcast_to([P, 8, BC])
            meanB = meanb[:].unsqueeze(2).broadcast_to([P, 8, BC])
            V.tensor_tensor(t0[:], xs[:], stdB, Alu.mult)
            V.tensor_tensor(x0s[:], t0[:], meanB, Alu.add)
            px, py, th, vv = x0s[:, 0, :], x0s[:, 1, :], x0s[:, 2, :], x0s[:, 3, :]
            oppx, oppy = x0s[:, 4, :], x0s[:, 5, :]

            # sin/cos with range wrap into [-pi, pi] (2 rounds, covers +-5pi)
            st = scr.tile([P, BC], f32)
            ct = scr.tile([P, BC], f32)
            w1t = scr.tile([P, BC], f32)
            w2t = scr.tile([P, BC], f32)
            w3t = scr.tile([P, BC], f32)

            def wrap_to(dst_ap, src_ap):
                cur = src_ap
                for _ in range(2):
                    V.tensor_scalar(w1t[:], cur, -PI, 2 * PI, Alu.is_lt, Alu.mult)
                    V.tensor_scalar(w2t[:], cur, PI, -2 * PI, Alu.is_gt, Alu.mult)
                    V.tensor_tensor(w1t[:], w1t[:], w2t[:], Alu.add)
                    V.tensor_tensor(dst_ap, w1t[:], cur, Alu.add)
                    cur = dst_ap

            wrap_to(w3t[:], th)
            nc.scalar.activation(st[:], w3t[:], AF.Sin)
            V.tensor_scalar(w3t[:], th, PI / 2, None, Alu.add)
            wrap_to(w3t[:], w3t[:])
            nc.scalar.activation(ct[:], w3t[:], AF.Sin)

            # dx, dy  [P, M, BC]
            dxP = scr.tile([P, M, BC], f32)
            dyP = scr.tile([P, M, BC], f32)
            pxB = px.unsqueeze(1).broadcast_to([P, 8, BC])
            pyB = py.unsqueeze(1).broadcast_to([P, 8, BC])
            oxB = obsb[:, 0, :].unsqueeze(2).broadcast_to([P, 8, BC])
            oyB = obsb[:, 1, :].unsqueeze(2).broadcast_to([P, 8, BC])
            V.scalar_tensor_tensor(dxP[:, 0:8, :], pxB, 1.0, oxB, Alu.mult, Alu.subtract)
            V.scalar_tensor_tensor(dyP[:, 0:8, :], pyB, 1.0, oyB, Alu.mult, Alu.subtract)
            V.tensor_tensor(dxP[:, 8, :], px, oppx, Alu.subtract)
            V.tensor_tensor(dyP[:, 8, :], py, oppy, Alu.subtract)

            # barrier = dx^2 + dy^2 - R^2
            bb3 = scr.tile([P, 3, M, BC], f32, name="bb3")
            V.memset(bb3[:, 2], 1.0)
            bar = bb3[:, 1]
            sq1 = scr.tile([P, M, BC], f32)
            V.tensor_tensor(sq1[:], dxP[:], dxP[:], Alu.mult)
            V.tensor_tensor(bar[:], dyP[:], dyP[:], Alu.mult)
            V.tensor_tensor(sq1[:], sq1[:], bar[:], Alu.add)   # dx^2+dy^2
            R2s = scr.tile([P, 8, BC], f32, name="R2s")
            orB = obsb[:, 2, :].unsqueeze(2).broadcast_to([P, 8, BC])
            V.tensor_scalar(R2s[:], orB, 0.6, None, Alu.add)
            V.tensor_tensor(R2s[:], R2s[:], R2s[:], Alu.mult)
            V.tensor_tensor(bar[:, 0:8, :], sq1[:, 0:8, :], R2s[:], Alu.subtract)
            V.tensor_scalar(bar[:, 8, :], sq1[:, 8, :], R2_OPP, None, Alu.subtract)

            # trig/velocity products
            vst = scr.tile([P, BC], f32)
            vct = scr.tile([P, BC], f32)
            nct2 = scr.tile([P, BC], f32)
            nst2 = scr.tile([P, BC], f32)
            V.scalar_tensor_tensor(vst[:], vv, 2.0, st[:], Alu.mult, Alu.mult)
            V.scalar_tensor_tensor(vct[:], vv, 2.0, ct[:], Alu.mult, Alu.mult)
            V.tensor_scalar(nct2[:], ct[:], -2.0, None, Alu.mult)
            V.tensor_scalar(nst2[:], st[:], -2.0, None, Alu.mult)
            vstB = vst[:].unsqueeze(1).broadcast_to([P, M, BC])
            vctB = vct[:].unsqueeze(1).broadcast_to([P, M, BC])
            nct2B = nct2[:].unsqueeze(1).broadcast_to([P, M, BC])
            nst2B = nst2[:].unsqueeze(1).broadcast_to([P, M, BC])

            q1 = scr.tile([P, M, BC], f32)
            q2 = scr.tile([P, M, BC], f32)
            bdot = bb3[:, 0]
            V.tensor_tensor(q1[:], dxP[:], vctB, Alu.mult)
            V.tensor_tensor(q2[:], dyP[:], vstB, Alu.mult)
            V.tensor_tensor(bdot[:], q1[:], q2[:], Alu.add)

            V.tensor_tensor(q1[:], dxP[:], vstB, Alu.mult)
            V.tensor_tensor(q2[:], dyP[:], vctB, Alu.mult)
            V.tensor_tensor(gx_mb, q1[:], q2[:], Alu.subtract)  # G1
            V.tensor_tensor(q1[:], dxP[:], nct2B, Alu.mult)
            V.tensor_tensor(q2[:], dyP[:], nst2B, Alu.mult)
            V.tensor_tensor(gy_mb, q1[:], q2[:], Alu.add)       # G2

            # alpha = 1 / (sqrt(Sxx^2 + 2*Sxy^2 + Syy^2) + 1e-6)
            # (independent of the MLP heads -- overlaps the matmuls)
            Sxx = scr.tile([P, BC], f32)
            Syy = scr.tile([P, BC], f32)
            Sxy = scr.tile([P, BC], f32)
            V.tensor_tensor(q1[:], gx_mb, gx_mb, Alu.mult)
            V.tensor_reduce(Sxx[:], q1[:].transpose([0, 2, 1]), AX.X, Alu.add)
            V.tensor_tensor(q1[:], gy_mb, gy_mb, Alu.mult)
            V.tensor_reduce(Syy[:], q1[:].transpose([0, 2, 1]), AX.X, Alu.add)
            V.tensor_tensor(q1[:], gx_mb, gy_mb, Alu.mult)
            V.tensor_reduce(Sxy[:], q1[:].transpose([0, 2, 1]), AX.X, Alu.add)
            wsum = scr.tile([P, BC], f32)
            V.tensor_tensor(wsum[:], Sxx[:], Sxx[:], Alu.mult)
            V.scalar_tensor_tensor(w1t[:], Sxy[:], 2.0, Sxy[:], Alu.mult, Alu.mult)
            V.tensor_tensor(wsum[:], wsum[:], w1t[:], Alu.add)
            V.tensor_tensor(w1t[:], Syy[:], Syy[:], Alu.mult)
            V.tensor_tensor(wsum[:], wsum[:], w1t[:], Alu.add)
            alph = scr.tile([P, BC], f32)
            Linv = scr.tile([P, BC], f32)        # ||A||_F + 1e-6  (= 1/alpha)
            nc.scalar.activation(w2t[:], wsum[:], AF.Sqrt)
            V.tensor_scalar(Linv[:], w2t[:], 1e-6, None, Alu.add)
            V.reciprocal(alph[:], Linv[:])
            alphB = alph[:].unsqueeze(1).broadcast_to([P, M, BC])
            V.tensor_tensor(agq[:, :, :, 0], gx_mb, alphB, Alu.mult)
            V.tensor_tensor(agq[:, :, :, 1], gy_mb, alphB, Alu.mult)

            V.memset(S3[:, :, 2], 1.0)

            # per-jump constants aT = alpha*T, aTa = alpha^2*T and lf2b:
            # heads-INDEPENDENT, so issued before the heads wait (the DVE
            # drains its queue in program order).
            gn3 = scr.tile([P, 3, BC], f32, name="gn3")
            V.scalar_tensor_tensor(gn3[:, 2, :], vv, 2.0, vv, Alu.mult, Alu.mult)
            NJ = len(JUMPS)
            ATH = scr.tile([P, NJ, BC], f32, name="ATH")
            ATAH = scr.tile([P, NJ, BC], f32, name="ATAH")
            for ji, Tj in enumerate(JUMPS):
                V.tensor_scalar(ATH[:, ji, :], alph[:], float(Tj), None, Alu.mult)
                V.tensor_tensor(ATAH[:, ji, :], ATH[:, ji, :], alph[:], Alu.mult)

            # --- heads-dependent tail of the prep (critical path) ---
            # h = 2v^2 + 4(s1+s2)*bdot + 16*s1*s2*barrier;  q = G.p + h
            # bb3 = [bdot | bar | ones], gn3 = [A4 | B16 | lf2b]; the two
            # strands (pr3-h and G.p) are independent -> interleaved.
            hq = scr.tile([P, M, BC], f32)
            pr3 = scr.tile([P, 3, M, BC], f32, name="pr3")
            pB2 = pg[:, 0:2, :].unsqueeze(3).broadcast_to([P, 2, BC, M])
            V.tensor_tensor(gn3[:, 0, :], sg1, sg2, Alu.add)
            V.tensor_tensor(T[:], gxy[:], pB2, Alu.mult)
            V.tensor_scalar(gn3[:, 0, :], gn3[:, 0, :], 4.0, None, Alu.mult)
            V.scalar_tensor_tensor(gn3[:, 1, :], sg1, 16.0, sg2, Alu.mult, Alu.mult)
            V.tensor_tensor(q2[:].transpose([0, 2, 1]), T[:, 0], T[:, 1], Alu.add)
            V.tensor_tensor(pr3[:], bb3[:],
                            gn3[:].unsqueeze(2).broadcast_to([P, 3, M, BC]), Alu.mult)
            V.tensor_reduce(hq[:], pr3[:].transpose([0, 2, 3, 1]), AX.X, Alu.add)
            V.tensor_tensor(hq[:], q2[:], hq[:], Alu.add)      # hq := q vector
            V.tensor_tensor(agq[:, :, :, 2], hq[:], alphB, Alu.mult)

            # iteration 1 from z=0 reduces to z_1 = -alpha*q: initialize the
            # state directly.
            V.tensor_scalar(lam[:].transpose([0, 2, 1]), agq[:, :, :, 2],
                            -1.0, None, Alu.mult)

            # ---------------- fine iterations (2..F_FINE) ----------------
            # Two sample-halves interleaved so consecutive DVE ops are
            # independent (hides the per-op pipe-drain stall).
            HB = BC // 2
            halves = [slice(0, HB), slice(HB, BC)]
            lam_b2 = [lam[:, hs, :].unsqueeze(1).broadcast_to([P, 2, HB, M])
                      for hs in halves]
            s_bM = [S3[:, hs, :].unsqueeze(1).broadcast_to([P, M, HB, 3])
                    for hs in halves]
            for it in range(F_FINE - 1):
                for i, hs in enumerate(halves):
                    V.scalar_tensor_tensor(T[:, :, hs, :], lam_b2[i], 0.0,
                                           gxy[:, :, hs, :], Alu.max, Alu.mult)
                for i, hs in enumerate(halves):
                    V.tensor_reduce(S3[:, hs, 0:2].transpose([0, 2, 1]),
                                    T[:, :, hs, :], AX.X, Alu.add)
                for i, hs in enumerate(halves):
                    V.tensor_tensor(Z[:, :, hs, :], agq[:, :, hs, :], s_bM[i], Alu.mult)
                for i, hs in enumerate(halves):
                    V.tensor_reduce(Vt[:, hs, :].transpose([0, 2, 1]),
                                    Z[:, :, hs, :], AX.X, Alu.add)
                for i, hs in enumerate(halves):
                    V.scalar_tensor_tensor(lam[:, hs, :], lam[:, hs, :], 0.0,
                                           Vt[:, hs, :], Alu.max, Alu.subtract)

            # ---------------- composed jumps ----------------
            # views
            alph2 = alph[:].unsqueeze(1).broadcast_to([P, 2, BC])
            Linv2 = Linv[:].unsqueeze(1).broadcast_to([P, 2, BC])

            TP = qp.tile([P, 2, BC, M], f32, name="TP")
            GM = qp.tile([P, 2, BC, M], f32, name="GM")
            GS = qp.tile([P, BC, M], f32, name="GS")
            GR = qp.tile([P, BC, M], f32, name="GR")
            SGm = qp.tile([P, BC, M], f32, name="SGm")
            Dlt = qp.tile([P, BC, M], f32, name="Dlt")
            GL = qp.tile([P, 2, BC], f32, name="GL")
            Sdg4 = qp.tile([P, 4, BC], f32, name="Sdg4")   # Sxx|Syy|Sxy|dd
            E3 = qp.tile([P, 3, BC], f32, name="E3")       # e1|e2|disc
            IED = qp.tile([P, 3, BC], f32, name="IED")     # 1/(e1+fl)|1/(e2+fl)|1/(disc+fl)
            SQ2 = qp.tile([P, 2, BC], f32, name="SQ2")
            F3 = qp.tile([P, 3, BC], f32, name="F3")
            GD = qp.tile([P, 2, BC], f32, name="GD")
            Se = qp.tile([P, 2, BC], f32, name="Se")
            Re = qp.tile([P, 2, BC], f32, name="Re")
            LNe = qp.tile([P, 2, BC], f32, name="LNe")
            EXe = qp.tile([P, 2, BC], f32, name="EXe")
            NUMe = qp.tile([P, 2, BC], f32, name="NUMe")
            PHI = qp.tile([P, 2, BC], f32, name="PHI")
            PSS = qp.tile([P, 2, BC], f32, name="PSS")
            PSI = qp.tile([P, 2, BC], f32, name="PSI")
            SER = qp.tile([P, 2, BC], f32, name="SER")
            WSL = qp.tile([P, 2, BC], f32, name="WSL")
            A1 = qp.tile([P, 2, BC], f32, name="A1")
            B1t = qp.tile([P, 2, BC], f32, name="B1t")
            Wv = qp.tile([P, 2, BC], f32, name="Wv")
            PWS = {k: qp.tile([P, 2, BC], f32, name=f"PW{k}")
                   for k in range(1, 8)}
            TRt = scr.tile([P, BC], f32, name="TRt")
            HTt = scr.tile([P, BC], f32, name="HTt")
            FLt = scr.tile([P, BC], f32, name="FLt")
            TH0 = scr.tile([P, BC], f32, name="TH0")
            TH1 = scr.tile([P, BC], f32, name="TH1")
            tA = scr.tile([P, BC], f32, name="tA")
            tB = scr.tile([P, BC], f32, name="tB")

            lamB = lam[:].unsqueeze(1).broadcast_to([P, 2, BC, M])
            sgB = SGm[:].unsqueeze(1).broadcast_to([P, 2, BC, M])
            dB = Dlt[:].unsqueeze(1).broadcast_to([P, 2, BC, M])
            wB = Wv[:].unsqueeze(3).broadcast_to([P, 2, BC, M])
            flB3 = FLt[:].unsqueeze(1).broadcast_to([P, 3, BC])
            th0B = TH0[:].unsqueeze(1).broadcast_to([P, 2, BC])
            th1B3 = TH1[:].unsqueeze(1).broadcast_to([P, 3, BC])
            phoB = F3[:, 2, :].unsqueeze(1).broadcast_to([P, 2, BC])

            sg_b2 = [SGm[:, hs, :].unsqueeze(1).broadcast_to([P, 2, HB, M])
                     for hs in halves]
            for ji, Tj in enumerate(JUMPS):
                Tf = float(Tj)
                ataB = ATAH[:, ji, :].unsqueeze(1).broadcast_to([P, 2, BC])
                atB = ATH[:, ji, :].unsqueeze(1).broadcast_to([P, 2, BC])
                # ---- alpha*grad via the agq 3-channel trick; the strictly
                # sequential M-chain runs as two interleaved sample-halves so
                # each dependent pair is separated by the other half's op.
                for i, hs in enumerate(halves):
                    V.scalar_tensor_tensor(T[:, :, hs, :], lam_b2[i], 0.0,
                                           gxy[:, :, hs, :], Alu.max, Alu.mult)
                for i, hs in enumerate(halves):
                    V.tensor_reduce(S3[:, hs, 0:2].transpose([0, 2, 1]),
                                    T[:, :, hs, :], AX.X, Alu.add)
                for i, hs in enumerate(halves):
                    V.tensor_tensor(Z[:, :, hs, :], agq[:, :, hs, :], s_bM[i],
                                    Alu.mult)
                for i, hs in enumerate(halves):
                    V.tensor_reduce(Vt[:, hs, :].transpose([0, 2, 1]),
                                    Z[:, :, hs, :], AX.X, Alu.add)
                # sigma = (lam>0)|(grad<0) via min(-z, a*grad) < 0
                for i, hs in enumerate(halves):
                    V.scalar_tensor_tensor(GS[:, hs, :], lam[:, hs, :], -1.0,
                                           Vt[:, hs, :], Alu.mult, Alu.min)
                for i, hs in enumerate(halves):
                    V.tensor_scalar(SGm[:, hs, :], GS[:, hs, :], 0.0, None,
                                    Alu.is_lt)
                for i, hs in enumerate(halves):
                    V.tensor_tensor(GM[:, :, hs, :], gxy[:, :, hs, :], sg_b2[i],
                                    Alu.mult)
                for i, hs in enumerate(halves):
                    V.scalar_tensor_tensor(Dlt[:, hs, :], Vt[:, hs, :], -1.0,
                                           SGm[:, hs, :], Alu.mult, Alu.mult)
                # ---- masked Ghat; disc chain first so Sqrt issues early ----
                V.tensor_tensor(TP[:], GM[:], GM[:], Alu.mult)
                V.tensor_tensor(GS[:], GM[:, 0], GM[:, 1], Alu.mult)
                V.tensor_reduce(Sdg4[:, 0:2, :], TP[:], AX.X, Alu.add)  # Sxx|Syy
                V.tensor_reduce(Sdg4[:, 2, :], GS[:], AX.X, Alu.add)    # Sxy
                V.tensor_tensor(Sdg4[:, 3, :], Sdg4[:, 0, :], Sdg4[:, 1, :],
                                Alu.subtract)                           # dd
                V.tensor_tensor(SQ2[:], Sdg4[:, 2:4, :], Sdg4[:, 2:4, :], Alu.mult)
                V.scalar_tensor_tensor(tA[:], SQ2[:, 0], 4.0, SQ2[:, 1],
                                       Alu.mult, Alu.add)               # disc^2
                nc.scalar.activation(E3[:, 2, :], tA[:], AF.Sqrt)
                # (fill Sqrt latency with independent work)
                V.tensor_tensor(TP[:], GM[:], dB, Alu.mult)            # Gm*delta
                V.tensor_tensor(TRt[:], Sdg4[:, 0, :], Sdg4[:, 1, :], Alu.add)
                V.tensor_scalar(FLt[:], TRt[:], 1e-6, 1e-12, Alu.mult, Alu.add)
                V.tensor_scalar(HTt[:], TRt[:], 0.5, None, Alu.mult)
                # ---- eigenvalues / reciprocals (packed e1|e2|disc) ----
                V.scalar_tensor_tensor(E3[:, 0, :], E3[:, 2, :], 0.5, HTt[:],
                                       Alu.mult, Alu.add)
                V.scalar_tensor_tensor(E3[:, 1, :], E3[:, 2, :], -0.5, HTt[:],
                                       Alu.mult, Alu.add)
                V.tensor_tensor(IED[:], E3[:], flB3, Alu.add)
                V.tensor_tensor(Se[:], E3[:, 0:2, :], alph2, Alu.mult)
                V.reciprocal(IED[:], IED[:])
                V.tensor_scalar(Re[:], Se[:], -1.0, 1.0, Alu.mult, Alu.add)
                # r^T by repeated squaring on the DVE (r in [0,1]): the ACT
                # Ln/Exp route costs two 1.28us ACT_TABLE_LOADs per jump.
                # Weave the series strand + gdelta reduce into the chain.
                c1s = (Tf - 1.0) / 2.0
                c2s = (Tf - 1.0) * (Tf - 2.0) / 6.0
                c3s = (Tf - 1.0) * (Tf - 2.0) * (Tf - 3.0) / 24.0
                bits = [k for k in range(Tj.bit_length()) if (Tj >> k) & 1]
                series_ops = [
                    lambda: V.tensor_scalar(SER[:], Se[:], c3s, -c2s, Alu.mult, Alu.add),
                    lambda: V.tensor_scalar(WSL[:], Se[:], Tf, 0.1, Alu.mult, Alu.is_lt),
                    lambda: V.tensor_tensor(SER[:], SER[:], Se[:], Alu.mult),
                    lambda: V.tensor_scalar(SER[:], SER[:], c1s, None, Alu.add),
                    lambda: V.tensor_tensor(PSS[:], SER[:], ataB, Alu.mult),
                    lambda: V.tensor_reduce(GD[:], TP[:], AX.X, Alu.add),
                ]
                si = 0
                PW = {0: Re}
                for k in range(1, bits[-1] + 1):
                    V.tensor_tensor(PWS[k][:], PW[k - 1][:], PW[k - 1][:], Alu.mult)
                    PW[k] = PWS[k]
                    if si < len(series_ops):
                        series_ops[si](); si += 1
                acc = PW[bits[-1]]
                for i, k in enumerate(reversed(bits[:-1])):
                    dst = EXe if i % 2 == 0 else LNe
                    V.tensor_tensor(dst[:], acc[:], PW[k][:], Alu.mult)
                    if si < len(series_ops):
                        series_ops[si](); si += 1
                    acc = dst
                while si < len(series_ops):
                    series_ops[si](); si += 1
                EXr = acc           # holds r^T
                # ---- psi -> theta ----
                V.tensor_scalar(NUMe[:], EXr[:], -1.0, 1.0, Alu.mult, Alu.add)
                V.tensor_tensor(PHI[:], NUMe[:], IED[:, 0:2, :], Alu.mult)
                V.tensor_tensor(PSI[:], atB, PHI[:], Alu.subtract)
                V.tensor_tensor(PSI[:], PSI[:], IED[:, 0:2, :], Alu.mult)
                # blend psi_series where s*T < 0.1
                V.tensor_tensor(PHI[:], PSS[:], PSI[:], Alu.subtract)
                V.tensor_tensor(PHI[:], PHI[:], WSL[:], Alu.mult)
                V.tensor_tensor(PSI[:], PSI[:], PHI[:], Alu.add)
                V.tensor_tensor(PSI[:], PSI[:], Linv2, Alu.mult)   # psi*Linv
                V.tensor_tensor(tA[:], PSI[:, 0], PSI[:, 1], Alu.subtract)
                V.tensor_tensor(TH1[:], tA[:], IED[:, 2, :], Alu.mult)
                V.tensor_tensor(tB[:], TH1[:], E3[:, 0, :], Alu.mult)
                V.tensor_tensor(F3[:], Sdg4[:, 0:3, :], th1B3, Alu.mult)
                V.tensor_tensor(TH0[:], PSI[:, 0], tB[:], Alu.subtract)
                V.tensor_tensor(F3[:, 0:2, :], F3[:, 0:2, :], th0B, Alu.add)
                V.tensor_tensor(B1t[:], GD[:], phoB, Alu.mult)     # [o*g1, o*g2]
                V.tensor_tensor(A1[:], F3[:, 0:2, :], GD[:], Alu.mult)
                V.tensor_tensor(Wv[:, 0], A1[:, 0], B1t[:, 1], Alu.add)
                V.tensor_tensor(Wv[:, 1], A1[:, 1], B1t[:, 0], Alu.add)
                # ---- apply: z' = relu(z) + T*delta - Gm w ----
                for i, hs in enumerate(halves):
                    V.tensor_tensor(TP[:, :, hs, :], GM[:, :, hs, :],
                                    Wv[:, :, hs].unsqueeze(3).broadcast_to(
                                        [P, 2, HB, M]), Alu.mult)
                for i, hs in enumerate(halves):
                    V.tensor_tensor(GS[:, hs, :], TP[:, 0, hs, :],
                                    TP[:, 1, hs, :], Alu.add)
                for i, hs in enumerate(halves):
                    V.scalar_tensor_tensor(GR[:, hs, :], Dlt[:, hs, :], Tf,
                                           GS[:, hs, :], Alu.mult, Alu.subtract)
                for i, hs in enumerate(halves):
                    V.scalar_tensor_tensor(lam[:, hs, :], lam[:, hs, :], 0.0,
                                           GR[:, hs, :], Alu.max, Alu.add)

            # ---------------- u = -p - G^T relu(lam) ----------------
            V.scalar_tensor_tensor(TP[:], lamB, 0.0, gxy[:], Alu.max, Alu.mult)
            V.tensor_reduce(GL[:], TP[:], AX.X, Alu.add)
            u12 = scr.tile([P, BC, 2], f32)
            V.scalar_tensor_tensor(u12[:, :, 0], GL[:, 0], -1.0, p1c, Alu.mult, Alu.subtract)
            V.scalar_tensor_tensor(u12[:, :, 1], GL[:, 1], -1.0, p2c, Alu.mult, Alu.subtract)
            nc.sync.dma_start(y_d.rearrange("(b p) c -> p b c", p=P), u12[:])

    nc.finalize()
    _split_multi_waits(nc)
    return nc


_CACHED = {}


def _get_kernel():
    if "nc" not in _CACHED:
        _CACHED["nc"] = build_kernel()
    return _CACHED["nc"]


def _round_tf32(a):
    """RNE to 10-bit mantissa (TF32) so f32r consumers see pre-rounded data."""
    v = np.ascontiguousarray(np.asarray(a, np.float32)).view(np.uint32)
    r = v + np.uint32(0xFFF) + ((v >> np.uint32(13)) & np.uint32(1))
    r &= np.uint32(0xFFFFE000)
    return r.view(np.float32)


def build_in_maps(inputs):
    x = np.ascontiguousarray(np.asarray(inputs["x"], dtype=np.float32))
    obstacles = np.asarray(inputs["obstacles"], dtype=np.float32)
    std = np.asarray(inputs["std"], dtype=np.float32)
    mean = np.asarray(inputs["mean"], dtype=np.float32)

    rw = _round_tf32
    shared = {
        "W1": rw(inputs["W1"]),
        "b1": np.ascontiguousarray(np.asarray(inputs["b1"], np.float32)),
        "W21": rw(inputs["W21"]),
        "b21": np.ascontiguousarray(np.asarray(inputs["b21"], np.float32)),
        "W22": rw(inputs["W22"]),
        "b22": np.ascontiguousarray(np.asarray(inputs["b22"], np.float32)),
        "W31": rw(inputs["W31"]),
        "b31": np.ascontiguousarray(np.asarray(inputs["b31"], np.float32)),
        "W32": rw(inputs["W32"]),
        "b32": np.ascontiguousarray(np.asarray(inputs["b32"], np.float32)),
        "id4": np.eye(4, dtype=np.float32),
        "obsb": np.ascontiguousarray(
            np.broadcast_to(obstacles.T[None, :, :], (P, 3, 8)).astype(np.float32)),
        "stdb": np.ascontiguousarray(np.broadcast_to(std[None, :], (P, 8))),
        "meanb": np.ascontiguousarray(np.broadcast_to(mean[None, :], (P, 8))),
    }

    in_maps = []
    for c in range(N_CORES):
        xe = x[c * B_CORE:(c + 1) * B_CORE]            # [1024, 8]
        m = dict(shared)
        m["xT"] = rw(xe.T)                             # [8, 1024] (TF32-rounded)
        # sample-layout gather for the barrier math:
        m["xsg"] = np.ascontiguousarray(
            xe.reshape(BC, P, 8).transpose(1, 2, 0))   # [p, f, b]
        in_maps.append(m)
    return in_maps


def kernel(**inputs):
    in_maps = build_in_maps(inputs)
    nc = _get_kernel()
    res = run_bass_kernel_spmd(nc, in_maps, core_ids=list(range(N_CORES)))
    out = np.concatenate([res.results[c]["y"] for c in range(N_CORES)], axis=0)
    return out.astype(np.float32)


# revision 33
# speedup vs baseline: 1.0937x; 1.0558x over previous
"""BarrierNet Trainium2 kernel.

Data-parallel over 8 NeuronCores: batch 8192 -> 1024 samples/core.

Per core:
  * MLP (x @ W1 -> relu -> 2 branches -> heads) on the TensorEngine in
    f32r (1 cycle/row at >=256-col moving), bias+relu/sigmoid fused into
    ScalarEngine activations reading PSUM.  Heads land as 2x [2, B]
    (c-on-partition) and are transposed to sample-major [P, 4, BC] with
    PE identity-matmul transposes into one PSUM tile + a single DVE copy
    (replaces a ~12us DRAM roundtrip).
  * QP: the reference's 300 projected-gradient-ascent dual iterations are
    reproduced exactly-enough by 1 closed-form iteration
    (z1 = -alpha*q) + 3 composed "jump" steps [51, 96, 152]:
      - A = G G^T has rank <= 2 through G ([m,2]); T un-clipped steps
        compose to lam_T = lam + T*delta - Gm Theta Gm^T delta, where
        delta = -alpha*sigma.grad is one masked fine step and
        Theta = th0*I + th1*Ghat is an analytic 2x2 function of
        Ghat = Gm^T Gm evaluated from its eigenvalues e1/e2:
        psi(e) = (alpha T - phi(e))/e, phi(e) = (1-(1-alpha e)^T)/e,
        with a Taylor branch blended in where alpha*e*T < 0.1 (exact-0
        eigenvalues are the common rank<=1 case).
      - clipped coordinates must leave the linear dynamics: mask
        sigma = (lam>0)|(grad<0) (as min(-z, a*grad)<0 on the pre-clip
        state z), Gm = sigma*G, refreshed at every jump boundary; the
        boundary clips catch mid-course constraint absorptions within
        <=2x of onset.  Validated offline in strict fp32: 6.2e-3 rel err
        vs the fp32 reference (gate 2e-2); measured on-device identically.
      - (1-alpha e)^T via DVE repeated squaring: the ACT Ln/Exp route
        costs two 1.28us ACT_TABLE_LOADs per jump.
      - coefficient math packed into multi-channel tiles ([Sxx|Syy|Sxy|dd],
        [e1|e2|disc] with one fused reciprocal, 3-channel Phi) and woven
        so dependent DVE ops are separated (dependent cadence ~200ns vs
        ~84ns issue-limited).
  * Barrier/G/alpha prep and all DMAs except W22 overlap the MLP; the
    heads-dependent tail (q assembly, packed 3-channel products) is the
    only pre-jump critical path.
"""

import numpy as np

import concourse.bass as bass
import concourse.mybir as mybir
import concourse.tile as tile
from concourse.bass_utils import run_bass_kernel_spmd

f32 = mybir.dt.float32
AF = mybir.ActivationFunctionType
Alu = mybir.AluOpType
AX = mybir.AxisListType

# f32r (TF32-like) for the MLP matmuls: at >=256-col moving it runs at the
# PE's 1 cycle/row peak (same as fp16) without cast precision loss.
MLP_DT = mybir.dt.float32r

N_CORES = 8
B_TOTAL = 8192
B_CORE = B_TOTAL // N_CORES          # 1024
P = 128                              # partitions
BC = B_CORE // P                     # 8 b-chunks
M = 9                                # 8 static obstacles + opponent
PI = float(np.pi)
R2_OPP = float(np.float32(1.1) * np.float32(1.1))  # (0.5+0.5+0.1)^2 in f32

F_FINE = 1                           # init counts as iteration 1
JUMPS = [51, 96, 152]                # sum = 299 = 300 - F_FINE


def _split_multi_waits(nc, max_waits=1):
    """This walrus build only supports one sync-wait command per
    instruction.  Move excess waits onto preceding same-engine NOPs."""
    uid = [0]
    for fn in nc.m.functions:
        for blk in fn.blocks:
            insts = blk.instructions
            new = []
            for ins in insts:
                si = getattr(ins, "sync_info", None)
                waits = list(si.on_wait) if (si is not None and si.on_wait) else []
                if len(waits) > max_waits:
                    rest = waits[max_waits:]
                    for i in range(0, len(rest), max_waits):
                        uid[0] += 1
                        new.append(mybir.InstNoOp(
                            name=f"wsplit_{uid[0]}",
                            engine=ins.engine,
                            bass_nofuse=True,
                            sync_info=mybir.SyncInfo(
                                on_wait=rest[i:i + max_waits], on_update=[]),
                        ))
                    ins.sync_info = mybir.SyncInfo(
                        on_wait=waits[:max_waits],
                        on_update=list(si.on_update) if si.on_update else [])
                new.append(ins)
            blk.instructions = new


def build_kernel():
    nc = bass.Bass()

    # ---- DRAM I/O (per core) ----
    xT_d = nc.dram_tensor("xT", (8, B_CORE), f32, kind="ExternalInput")
    W1_d = nc.dram_tensor("W1", (8, 1024), f32, kind="ExternalInput")
    b1_d = nc.dram_tensor("b1", (1024,), f32, kind="ExternalInput")
    W21_d = nc.dram_tensor("W21", (1024, 512), f32, kind="ExternalInput")
    b21_d = nc.dram_tensor("b21", (512,), f32, kind="ExternalInput")
    W22_d = nc.dram_tensor("W22", (1024, 512), f32, kind="ExternalInput")
    b22_d = nc.dram_tensor("b22", (512,), f32, kind="ExternalInput")
    W31_d = nc.dram_tensor("W31", (512, 2), f32, kind="ExternalInput")
    b31_d = nc.dram_tensor("b31", (2,), f32, kind="ExternalInput")
    W32_d = nc.dram_tensor("W32", (512, 2), f32, kind="ExternalInput")
    b32_d = nc.dram_tensor("b32", (2,), f32, kind="ExternalInput")
    xsg_d = nc.dram_tensor("xsg", (P, 8, BC), f32, kind="ExternalInput")
    obsb_d = nc.dram_tensor("obsb", (P, 3, 8), f32, kind="ExternalInput")
    stdb_d = nc.dram_tensor("stdb", (P, 8), f32, kind="ExternalInput")
    meanb_d = nc.dram_tensor("meanb", (P, 8), f32, kind="ExternalInput")
    id4_d = nc.dram_tensor("id4", (4, 4), f32, kind="ExternalInput")
    y_d = nc.dram_tensor("y", (P, BC, 2), f32, kind="ExternalOutput")

    with tile.TileContext(nc) as tc:
        with (
            tc.tile_pool(name="w", bufs=1) as wp,
            tc.tile_pool(name="act", bufs=1) as ap,
            tc.tile_pool(name="qp", bufs=1) as qp,
            tc.tile_pool(name="scr", bufs=1) as scr,
            tc.tile_pool(name="ps", bufs=6, space="PSUM") as ps,
            tc.tile_pool(name="psh", bufs=2, space="PSUM") as psh,
        ):
            # ---------------- load ----------------
            xT = wp.tile([8, B_CORE], f32)
            W1 = wp.tile([8, 1024], f32)
            b1 = wp.tile([P, 8], f32)          # b1[p, mo] = b1_d[mo*128+p]
            W21 = wp.tile([P, 8, 512], f32)    # [p, k, n] = W21_d[k*128+p, n]
            W22 = wp.tile([P, 8, 512], f32)
            b2 = wp.tile([P, 2, 4], f32)       # [p, j, mo] = b2j_d[mo*128+p]
            W31 = wp.tile([P, 4, 2], f32)      # [p, kk, c] = W31_d[kk*128+p, c]
            W32 = wp.tile([P, 4, 2], f32)
            b3 = wp.tile([2, 2], f32)          # [c, j]: b31 | b32
            id4 = wp.tile([4, 4], f32)
            obsb = wp.tile([P, 3, 8], f32)
            stdb = wp.tile([P, 8], f32)
            meanb = wp.tile([P, 8], f32)
            xs = wp.tile([P, 8, BC], f32)      # [p, f, b] = x[b*128+p, f]

            nc.sync.dma_start(xT[:], xT_d[:])
            nc.sync.dma_start(W1[:], W1_d[:])
            nc.sync.dma_start(b1[:], b1_d.rearrange("(mo p) -> p mo", p=P))
            # W21 first (L2's k-accumulation needs all 8 chunks), then the
            # small prep tensors, then W22 (only needed at L2's midpoint)
            for k in range(8):
                nc.sync.dma_start(W21[:, k, :],
                                  W21_d.rearrange("(k p) n -> p k n", p=P)[:, k, :])
            nc.sync.dma_start(xs[:], xsg_d[:])
            nc.sync.dma_start(obsb[:], obsb_d[:])
            nc.sync.dma_start(stdb[:], stdb_d[:])
            nc.sync.dma_start(meanb[:], meanb_d[:])
            nc.sync.dma_start(b2[:, 0, :], b21_d.rearrange("(mo p) -> p mo", p=P))
            nc.sync.dma_start(b2[:, 1, :], b22_d.rearrange("(mo p) -> p mo", p=P))
            nc.sync.dma_start(W31[:], W31_d.rearrange("(kk p) c -> p kk c", p=P))
            nc.sync.dma_start(W32[:], W32_d.rearrange("(kk p) c -> p kk c", p=P))
            nc.sync.dma_start(b3[:, 0], b31_d[:].unsqueeze(0).transpose([1, 0]))
            nc.sync.dma_start(b3[:, 1], b32_d[:].unsqueeze(0).transpose([1, 0]))
            nc.sync.dma_start(id4[:], id4_d[:])
            for k in range(8):
                nc.sync.dma_start(W22[:, k, :],
                                  W22_d.rearrange("(k p) n -> p k n", p=P)[:, k, :])

            # ---------------- MLP operand casts ----------------
            W1r = wp.tile([8, 1024], MLP_DT, name="W1r")
            xTr = wp.tile([8, B_CORE], MLP_DT, name="xTr")
            W21r = wp.tile([P, 8, 512], MLP_DT, name="W21r")
            W22r = wp.tile([P, 8, 512], MLP_DT, name="W22r")
            W31r = wp.tile([P, 4, 2], MLP_DT, name="W31r")
            W32r = wp.tile([P, 4, 2], MLP_DT, name="W32r")
            # halves so the first L1 matmul starts after half the cast work
            nc.vector.tensor_copy(xTr[:, 0:512], xT[:, 0:512])
            nc.vector.tensor_copy(W1r[:, 0:512], W1[:, 0:512])
            nc.vector.tensor_copy(xTr[:, 512:1024], xT[:, 512:1024])
            nc.vector.tensor_copy(W1r[:, 512:1024], W1[:, 512:1024])
            for k in range(8):
                nc.vector.tensor_copy(W21r[:, k, :], W21[:, k, :])
                nc.vector.tensor_copy(W22r[:, k, :], W22[:, k, :])
            nc.vector.tensor_copy(W31r[:], W31[:])
            nc.vector.tensor_copy(W32r[:], W32[:])

            # L1 evacuation stays on ACT only: routing half to the DVE was
            # measured to push the prep chain (same DVE queue) 3.5us later,
            # delaying the loop start.
            NH = 512  # moving free dim per matmul
            h1T = ap.tile([P, 8, B_CORE], MLP_DT)     # [p, mo, n] : h1^T
            for mo in range(8):
                for hf in range(B_CORE // NH):
                    pt = ps.tile([P, NH], f32, name="ps_mm")
                    nc.tensor.matmul(pt[:], W1r[:, bass.ts(mo, P)],
                                     xTr[:, bass.ts(hf, NH)], start=True, stop=True)
                    nc.scalar.activation(h1T[:, mo, bass.ts(hf, NH)], pt[:],
                                         AF.Relu, bias=b1[:, mo:mo + 1])

            # L2 PSUM evacuation alternates ACT / DVE: the ACT engine's
            # (172+512)-cycle errata makes it the MLP's co-bottleneck, and
            # the DVE has slack here.  DVE path: relu(psum + bias) via
            # tensor_scalar (scalar ops run 2x for fp32).
            x2T = ap.tile([P, 2, 4, B_CORE], MLP_DT)  # [p, branch, mo, n]
            evac = 0
            for j, W2 in ((0, W21r), (1, W22r)):
                for mo in range(4):
                    for hf in range(B_CORE // NH):
                        pt = ps.tile([P, NH], f32, name="ps_mm")
                        for k in range(8):
                            nc.tensor.matmul(pt[:], W2[:, k, bass.ts(mo, P)],
                                             h1T[:, k, bass.ts(hf, NH)],
                                             start=(k == 0), stop=(k == 7))
                        dst = x2T[:, j, mo, bass.ts(hf, NH)]
                        if evac % 2 == 0:
                            nc.scalar.activation(dst, pt[:], AF.Relu,
                                                 bias=b2[:, j, mo:mo + 1])
                        else:
                            nc.vector.tensor_scalar(dst, pt[:],
                                                    b2[:, j, mo:mo + 1], 0.0,
                                                    Alu.add, Alu.max)
                        evac += 1

            # heads -> h3x [2, B_CORE] (c on partitions): h31 = x31,
            # h32 = sigmoid(x32-preact)
            h31 = ap.tile([2, B_CORE], f32, name="h31")
            h32 = ap.tile([2, B_CORE], f32, name="h32")
            for j, W3 in ((0, W31r), (1, W32r)):
                dsts = (h31, h32)[j]
                for hf in range(B_CORE // NH):
                    pt2 = psh.tile([2, NH], f32, name="ps_hd")
                    for kk in range(4):
                        nc.tensor.matmul(pt2[:], W3[:, kk, :],
                                         x2T[:, j, kk, bass.ts(hf, NH)],
                                         start=(kk == 0), stop=(kk == 3))
                    func = AF.Identity if j == 0 else AF.Sigmoid
                    nc.scalar.activation(dsts[:, bass.ts(hf, NH)],
                                         pt2[:], func, bias=b3[:, j:j + 1])

            # PE transpose to sample layout: pg[p, ch, b], ch = [p1, p2, sg1, sg2]
            # All 16 transposes land in one PSUM tile -> single DVE copy.
            pg = wp.tile([P, 4, BC], f32)
            ptT = ps.tile([P, 4 * BC], f32, name="ps_mm")
            for hf in range(BC):
                nc.tensor.matmul(ptT[:, 4 * hf:4 * hf + 2],
                                 h31[:, bass.ts(hf, P)], id4[0:2, 0:2],
                                 is_transpose=True)
                nc.tensor.matmul(ptT[:, 4 * hf + 2:4 * hf + 4],
                                 h32[:, bass.ts(hf, P)], id4[0:2, 0:2],
                                 is_transpose=True)
            nc.vector.tensor_copy(
                pg[:], ptT[:].rearrange("p (b c) -> p c b", c=4))
            p1c, p2c = pg[:, 0, :], pg[:, 1, :]
            sg1, sg2 = pg[:, 2, :], pg[:, 3, :]

            # ---------------- barrier / QP prep ----------------
            # Everything up to (and including) alpha depends only on x/obsb
            # DMAs, so it runs on the DVE while the PE is still in the MLP.
            V = nc.vector
            gxy = qp.tile([P, 2, BC, M], f32)    # Gx | Gy
            agq = qp.tile([P, M, BC, 3], f32)    # aGx | aGy | aq
            lam = qp.tile([P, BC, M], f32)       # pre-clip dual state z
            S3 = qp.tile([P, BC, 3], f32)
            T = qp.tile([P, 2, BC, M], f32)
            Z = qp.tile([P, M, BC, 3], f32)
            Vt = qp.tile([P, BC, M], f32)
            # [m, b]-ordered views for the prep ops
            gx_mb = gxy[:, 0, :, :].transpose([0, 2, 1])   # [P, M, BC]
            gy_mb = gxy[:, 1, :, :].transpose([0, 2, 1])

            x0s = scr.tile([P, 8, BC], f32)      # un-normalized state
            t0 = scr.tile([P, 8, BC], f32)
            stdB = stdb[:].unsqueeze(2).broadcast_to([P, 8, BC])
            meanB = meanb[:].unsqueeze(2).broadcast_to([P, 8, BC])
            V.tensor_tensor(t0[:], xs[:], stdB, Alu.mult)
            V.tensor_tensor(x0s[:], t0[:], meanB, Alu.add)
            px, py, th, vv = x0s[:, 0, :], x0s[:, 1, :], x0s[:, 2, :], x0s[:, 3, :]
            oppx, oppy = x0s[:, 4, :], x0s[:, 5, :]

            # sin/cos with range wrap into [-pi, pi] (2 rounds, covers +-5pi)
            st = scr.tile([P, BC], f32)
            ct = scr.tile([P, BC], f32)
            w1t = scr.tile([P, BC], f32)
            w2t = scr.tile([P, BC], f32)
            w3t = scr.tile([P, BC], f32)

            def wrap_to(dst_ap, src_ap):
                cur = src_ap
                for _ in range(2):
                    V.tensor_scalar(w1t[:], cur, -PI, 2 * PI, Alu.is_lt, Alu.mult)
                    V.tensor_scalar(w2t[:], cur, PI, -2 * PI, Alu.is_gt, Alu.mult)
                    V.tensor_tensor(w1t[:], w1t[:], w2t[:], Alu.add)
                    V.tensor_tensor(dst_ap, w1t[:], cur, Alu.add)
                    cur = dst_ap

            wrap_to(w3t[:], th)
            nc.scalar.activation(st[:], w3t[:], AF.Sin)
            V.tensor_scalar(w3t[:], th, PI / 2, None, Alu.add)
            wrap_to(w3t[:], w3t[:])
            nc.scalar.activation(ct[:], w3t[:], AF.Sin)

            # dx, dy  [P, M, BC]
            dxP = scr.tile([P, M, BC], f32)
            dyP = scr.tile([P, M, BC], f32)
            pxB = px.unsqueeze(1).broadcast_to([P, 8, BC])
            pyB = py.unsqueeze(1).broadcast_to([P, 8, BC])
            oxB = obsb[:, 0, :].unsqueeze(2).broadcast_to([P, 8, BC])
            oyB = obsb[:, 1, :].unsqueeze(2).broadcast_to([P, 8, BC])
            V.scalar_tensor_tensor(dxP[:, 0:8, :], pxB, 1.0, oxB, Alu.mult, Alu.subtract)
            V.scalar_tensor_tensor(dyP[:, 0:8, :], pyB, 1.0, oyB, Alu.mult, Alu.subtract)
            V.tensor_tensor(dxP[:, 8, :], px, oppx, Alu.subtract)
            V.tensor_tensor(dyP[:, 8, :], py, oppy, Alu.subtract)

            # barrier = dx^2 + dy^2 - R^2
            bb3 = scr.tile([P, 3, M, BC], f32, name="bb3")
            V.memset(bb3[:, 2], 1.0)
            bar = bb3[:, 1]
            sq1 = scr.tile([P, M, BC], f32)
            V.tensor_tensor(sq1[:], dxP[:], dxP[:], Alu.mult)
            V.tensor_tensor(bar[:], dyP[:], dyP[:], Alu.mult)
            V.tensor_tensor(sq1[:], sq1[:], bar[:], Alu.add)   # dx^2+dy^2
            R2s = scr.tile([P, 8, BC], f32, name="R2s")
            orB = obsb[:, 2, :].unsqueeze(2).broadcast_to([P, 8, BC])
            V.tensor_scalar(R2s[:], orB, 0.6, None, Alu.add)
            V.tensor_tensor(R2s[:], R2s[:], R2s[:], Alu.mult)
            V.tensor_tensor(bar[:, 0:8, :], sq1[:, 0:8, :], R2s[:], Alu.subtract)
            V.tensor_scalar(bar[:, 8, :], sq1[:, 8, :], R2_OPP, None, Alu.subtract)

            # trig/velocity products
            vst = scr.tile([P, BC], f32)
            vct = scr.tile([P, BC], f32)
            nct2 = scr.tile([P, BC], f32)
            nst2 = scr.tile([P, BC], f32)
            V.scalar_tensor_tensor(vst[:], vv, 2.0, st[:], Alu.mult, Alu.mult)
            V.scalar_tensor_tensor(vct[:], vv, 2.0, ct[:], Alu.mult, Alu.mult)
            V.tensor_scalar(nct2[:], ct[:], -2.0, None, Alu.mult)
            V.tensor_scalar(nst2[:], st[:], -2.0, None, Alu.mult)
            vstB = vst[:].unsqueeze(1).broadcast_to([P, M, BC])
            vctB = vct[:].unsqueeze(1).broadcast_to([P, M, BC])
            nct2B = nct2[:].unsqueeze(1).broadcast_to([P, M, BC])
            nst2B = nst2[:].unsqueeze(1).broadcast_to([P, M, BC])

            q1 = scr.tile([P, M, BC], f32)
            q2 = scr.tile([P, M, BC], f32)
            bdot = bb3[:, 0]
            V.tensor_tensor(q1[:], dxP[:], vctB, Alu.mult)
            V.tensor_tensor(q2[:], dyP[:], vstB, Alu.mult)
            V.tensor_tensor(bdot[:], q1[:], q2[:], Alu.add)

            V.tensor_tensor(q1[:], dxP[:], vstB, Alu.mult)
            V.tensor_tensor(q2[:], dyP[:], vctB, Alu.mult)
            V.tensor_tensor(gx_mb, q1[:], q2[:], Alu.subtract)  # G1
            V.tensor_tensor(q1[:], dxP[:], nct2B, Alu.mult)
            V.tensor_tensor(q2[:], dyP[:], nst2B, Alu.mult)
            V.tensor_tensor(gy_mb, q1[:], q2[:], Alu.add)       # G2

            # alpha = 1 / (sqrt(Sxx^2 + 2*Sxy^2 + Syy^2) + 1e-6)
            # (independent of the MLP heads -- overlaps the matmuls)
            Sxx = scr.tile([P, BC], f32)
            Syy = scr.tile([P, BC], f32)
            Sxy = scr.tile([P, BC], f32)
            V.tensor_tensor(q1[:], gx_mb, gx_mb, Alu.mult)
            V.tensor_reduce(Sxx[:], q1[:].transpose([0, 2, 1]), AX.X, Alu.add)
            V.tensor_tensor(q1[:], gy_mb, gy_mb, Alu.mult)
            V.tensor_reduce(Syy[:], q1[:].transpose([0, 2, 1]), AX.X, Alu.add)
            V.tensor_tensor(q1[:], gx_mb, gy_mb, Alu.mult)
            V.tensor_reduce(Sxy[:], q1[:].transpose([0, 2, 1]), AX.X, Alu.add)
            wsum = scr.tile([P, BC], f32)
            V.tensor_tensor(wsum[:], Sxx[:], Sxx[:], Alu.mult)
            V.scalar_tensor_tensor(w1t[:], Sxy[:], 2.0, Sxy[:], Alu.mult, Alu.mult)
            V.tensor_tensor(wsum[:], wsum[:], w1t[:], Alu.add)
            V.tensor_tensor(w1t[:], Syy[:], Syy[:], Alu.mult)
            V.tensor_tensor(wsum[:], wsum[:], w1t[:], Alu.add)
            alph = scr.tile([P, BC], f32)
            Linv = scr.tile([P, BC], f32)        # ||A||_F + 1e-6  (= 1/alpha)
            nc.scalar.activation(w2t[:], wsum[:], AF.Sqrt)
            V.tensor_scalar(Linv[:], w2t[:], 1e-6, None, Alu.add)
            V.reciprocal(alph[:], Linv[:])
            alphB = alph[:].unsqueeze(1).broadcast_to([P, M, BC])
            V.tensor_tensor(agq[:, :, :, 0], gx_mb, alphB, Alu.mult)
            V.tensor_tensor(agq[:, :, :, 1], gy_mb, alphB, Alu.mult)

            V.memset(S3[:, :, 2], 1.0)

            # per-jump constants aT = alpha*T, aTa = alpha^2*T and lf2b:
            # heads-INDEPENDENT, so issued before the heads wait (the DVE
            # drains its queue in program order).
            gn3 = scr.tile([P, 3, BC], f32, name="gn3")
            V.scalar_tensor_tensor(gn3[:, 2, :], vv, 2.0, vv, Alu.mult, Alu.mult)
            NJ = len(JUMPS)
            ATH = scr.tile([P, NJ, BC], f32, name="ATH")
            ATAH = scr.tile([P, NJ, BC], f32, name="ATAH")
            for ji, Tj in enumerate(JUMPS):
                V.tensor_scalar(ATH[:, ji, :], alph[:], float(Tj), None, Alu.mult)
                V.tensor_tensor(ATAH[:, ji, :], ATH[:, ji, :], alph[:], Alu.mult)

            # --- heads-dependent tail of the prep (critical path) ---
            # h = 2v^2 + 4(s1+s2)*bdot + 16*s1*s2*barrier;  q = G.p + h
            # bb3 = [bdot | bar | ones], gn3 = [A4 | B16 | lf2b]; the two
            # strands (pr3-h and G.p) are independent -> interleaved.
            hq = scr.tile([P, M, BC], f32)
            pr3 = scr.tile([P, 3, M, BC], f32, name="pr3")
            pB2 = pg[:, 0:2, :].unsqueeze(3).broadcast_to([P, 2, BC, M])
            V.tensor_tensor(gn3[:, 0, :], sg1, sg2, Alu.add)
            V.tensor_tensor(T[:], gxy[:], pB2, Alu.mult)
            V.tensor_scalar(gn3[:, 0, :], gn3[:, 0, :], 4.0, None, Alu.mult)
            V.scalar_tensor_tensor(gn3[:, 1, :], sg1, 16.0, sg2, Alu.mult, Alu.mult)
            V.tensor_tensor(q2[:].transpose([0, 2, 1]), T[:, 0], T[:, 1], Alu.add)
            V.tensor_tensor(pr3[:], bb3[:],
                            gn3[:].unsqueeze(2).broadcast_to([P, 3, M, BC]), Alu.mult)
            V.tensor_reduce(hq[:], pr3[:].transpose([0, 2, 3, 1]), AX.X, Alu.add)
            V.tensor_tensor(hq[:], q2[:], hq[:], Alu.add)      # hq := q vector
            V.tensor_tensor(agq[:, :, :, 2], hq[:], alphB, Alu.mult)

            # iteration 1 from z=0 reduces to z_1 = -alpha*q: initialize the
            # state directly.
            V.tensor_scalar(lam[:].transpose([0, 2, 1]), agq[:, :, :, 2],
                            -1.0, None, Alu.mult)

            # ---------------- fine iterations (2..F_FINE) ----------------
            # Two sample-halves interleaved so consecutive DVE ops are
            # independent (hides the per-op pipe-drain stall).
            HB = BC // 2
            halves = [slice(0, HB), slice(HB, BC)]
            lam_b2 = [lam[:, hs, :].unsqueeze(1).broadcast_to([P, 2, HB, M])
                      for hs in halves]
            s_bM = [S3[:, hs, :].unsqueeze(1).broadcast_to([P, M, HB, 3])
                    for hs in halves]
            for it in range(F_FINE - 1):
                for i, hs in enumerate(halves):
                    V.scalar_tensor_tensor(T[:, :, hs, :], lam_b2[i], 0.0,
                                           gxy[:, :, hs, :], Alu.max, Alu.mult)
                for i, hs in enumerate(halves):
                    V.tensor_reduce(S3[:, hs, 0:2].transpose([0, 2, 1]),
                                    T[:, :, hs, :], AX.X, Alu.add)
                for i, hs in enumerate(halves):
                    V.tensor_tensor(Z[:, :, hs, :], agq[:, :, hs, :], s_bM[i], Alu.mult)
                for i, hs in enumerate(halves):
                    V.tensor_reduce(Vt[:, hs, :].transpose([0, 2, 1]),
                                    Z[:, :, hs, :], AX.X, Alu.add)
                for i, hs in enumerate(halves):
                    V.scalar_tensor_tensor(lam[:, hs, :], lam[:, hs, :], 0.0,
                                           Vt[:, hs, :], Alu.max, Alu.subtract)

            # ---------------- composed jumps ----------------
            # views
            alph2 = alph[:].unsqueeze(1).broadcast_to([P, 2, BC])
            Linv2 = Linv[:].unsqueeze(1).broadcast_to([P, 2, BC])

            TP = qp.tile([P, 2, BC, M], f32, name="TP")
            GM = qp.tile([P, 2, BC, M], f32, name="GM")
            GS = qp.tile([P, BC, M], f32, name="GS")
            GR = qp.tile([P, BC, M], f32, name="GR")
            SGm = qp.tile([P, BC, M], f32, name="SGm")
            Dlt = qp.tile([P, BC, M], f32, name="Dlt")
            GL = qp.tile([P, 2, BC], f32, name="GL")
            Sdg4 = qp.tile([P, 4, BC], f32, name="Sdg4")   # Sxx|Syy|Sxy|dd
            E3 = qp.tile([P, 3, BC], f32, name="E3")       # e1|e2|disc
            IED = qp.tile([P, 3, BC], f32, name="IED")     # 1/(e1+fl)|1/(e2+fl)|1/(disc+fl)
            SQ2 = qp.tile([P, 2, BC], f32, name="SQ2")
            F3 = qp.tile([P, 3, BC], f32, name="F3")
            GD = qp.tile([P, 2, BC], f32, name="GD")
            Se = qp.tile([P, 2, BC], f32, name="Se")
            Re = qp.tile([P, 2, BC], f32, name="Re")
            LNe = qp.tile([P, 2, BC], f32, name="LNe")
            EXe = qp.tile([P, 2, BC], f32, name="EXe")
            NUMe = qp.tile([P, 2, BC], f32, name="NUMe")
            PHI = qp.tile([P, 2, BC], f32, name="PHI")
            PSS = qp.tile([P, 2, BC], f32, name="PSS")
            PSI = qp.tile([P, 2, BC], f32, name="PSI")
            SER = qp.tile([P, 2, BC], f32, name="SER")
            WSL = qp.tile([P, 2, BC], f32, name="WSL")
            A1 = qp.tile([P, 2, BC], f32, name="A1")
            B1t = qp.tile([P, 2, BC], f32, name="B1t")
            Wv = qp.tile([P, 2, BC], f32, name="Wv")
            PWS = {k: qp.tile([P, 2, BC], f32, name=f"PW{k}")
                   for k in range(1, 8)}
            TRt = scr.tile([P, BC], f32, name="TRt")
            HTt = scr.tile([P, BC], f32, name="HTt")
            FLt = scr.tile([P, BC], f32, name="FLt")
            TH0 = scr.tile([P, BC], f32, name="TH0")
            TH1 = scr.tile([P, BC], f32, name="TH1")
            tA = scr.tile([P, BC], f32, name="tA")
            tB = scr.tile([P, BC], f32, name="tB")

            lamB = lam[:].unsqueeze(1).broadcast_to([P, 2, BC, M])
            sgB = SGm[:].unsqueeze(1).broadcast_to([P, 2, BC, M])
            dB = Dlt[:].unsqueeze(1).broadcast_to([P, 2, BC, M])
            wB = Wv[:].unsqueeze(3).broadcast_to([P, 2, BC, M])
            flB3 = FLt[:].unsqueeze(1).broadcast_to([P, 3, BC])
            th0B = TH0[:].unsqueeze(1).broadcast_to([P, 2, BC])
            th1B3 = TH1[:].unsqueeze(1).broadcast_to([P, 3, BC])
            phoB = F3[:, 2, :].unsqueeze(1).broadcast_to([P, 2, BC])

            sg_b2 = [SGm[:, hs, :].unsqueeze(1).broadcast_to([P, 2, HB, M])
                     for hs in halves]
            for ji, Tj in enumerate(JUMPS):
                Tf = float(Tj)
                ataB = ATAH[:, ji, :].unsqueeze(1).broadcast_to([P, 2, BC])
                atB = ATH[:, ji, :].unsqueeze(1).broadcast_to([P, 2, BC])
                # ---- alpha*grad via the agq 3-channel trick; the strictly
                # sequential M-chain runs as two interleaved sample-halves so
                # each dependent pair is separated by the other half's op.
                for i, hs in enumerate(halves):
                    V.scalar_tensor_tensor(T[:, :, hs, :], lam_b2[i], 0.0,
                                           gxy[:, :, hs, :], Alu.max, Alu.mult)
                for i, hs in enumerate(halves):
                    V.tensor_reduce(S3[:, hs, 0:2].transpose([0, 2, 1]),
                                    T[:, :, hs, :], AX.X, Alu.add)
                for i, hs in enumerate(halves):
                    V.tensor_tensor(Z[:, :, hs, :], agq[:, :, hs, :], s_bM[i],
                                    Alu.mult)
                for i, hs in enumerate(halves):
                    V.tensor_reduce(Vt[:, hs, :].transpose([0, 2, 1]),
                                    Z[:, :, hs, :], AX.X, Alu.add)
                # sigma = (lam>0)|(grad<0) via min(-z, a*grad) < 0
                for i, hs in enumerate(halves):
                    V.scalar_tensor_tensor(GS[:, hs, :], lam[:, hs, :], -1.0,
                                           Vt[:, hs, :], Alu.mult, Alu.min)
                for i, hs in enumerate(halves):
                    V.tensor_scalar(SGm[:, hs, :], GS[:, hs, :], 0.0, None,
                                    Alu.is_lt)
                for i, hs in enumerate(halves):
                    V.tensor_tensor(GM[:, :, hs, :], gxy[:, :, hs, :], sg_b2[i],
                                    Alu.mult)
                for i, hs in enumerate(halves):
                    V.scalar_tensor_tensor(Dlt[:, hs, :], Vt[:, hs, :], -1.0,
                                           SGm[:, hs, :], Alu.mult, Alu.mult)
                # ---- masked Ghat; disc chain first so Sqrt issues early ----
                V.tensor_tensor(TP[:], GM[:], GM[:], Alu.mult)
                V.tensor_tensor(GS[:], GM[:, 0], GM[:, 1], Alu.mult)
                V.tensor_reduce(Sdg4[:, 0:2, :], TP[:], AX.X, Alu.add)  # Sxx|Syy
                V.tensor_reduce(Sdg4[:, 2, :], GS[:], AX.X, Alu.add)    # Sxy
                V.tensor_tensor(Sdg4[:, 3, :], Sdg4[:, 0, :], Sdg4[:, 1, :],
                                Alu.subtract)                           # dd
                V.tensor_tensor(SQ2[:], Sdg4[:, 2:4, :], Sdg4[:, 2:4, :], Alu.mult)
                V.scalar_tensor_tensor(tA[:], SQ2[:, 0], 4.0, SQ2[:, 1],
                                       Alu.mult, Alu.add)               # disc^2
                nc.scalar.activation(E3[:, 2, :], tA[:], AF.Sqrt)
                # (fill Sqrt latency with independent work)
                V.tensor_tensor(TP[:], GM[:], dB, Alu.mult)            # Gm*delta
                V.tensor_tensor(TRt[:], Sdg4[:, 0, :], Sdg4[:, 1, :], Alu.add)
                V.tensor_scalar(FLt[:], TRt[:], 1e-6, 1e-12, Alu.mult, Alu.add)
                V.tensor_scalar(HTt[:], TRt[:], 0.5, None, Alu.mult)
                # ---- eigenvalues / reciprocals (packed e1|e2|disc) ----
                V.scalar_tensor_tensor(E3[:, 0, :], E3[:, 2, :], 0.5, HTt[:],
                                       Alu.mult, Alu.add)
                V.scalar_tensor_tensor(E3[:, 1, :], E3[:, 2, :], -0.5, HTt[:],
                                       Alu.mult, Alu.add)
                V.tensor_tensor(IED[:], E3[:], flB3, Alu.add)
                V.tensor_tensor(Se[:], E3[:, 0:2, :], alph2, Alu.mult)
                V.reciprocal(IED[:], IED[:])
                V.tensor_scalar(Re[:], Se[:], -1.0, 1.0, Alu.mult, Alu.add)
                # r^T by repeated squaring on the DVE (r in [0,1]): the ACT
                # Ln/Exp route costs two 1.28us ACT_TABLE_LOADs per jump.
                # Weave the series strand + gdelta reduce into the chain.
                c1s = (Tf - 1.0) / 2.0
                c2s = (Tf - 1.0) * (Tf - 2.0) / 6.0
                c3s = (Tf - 1.0) * (Tf - 2.0) * (Tf - 3.0) / 24.0
                bits = [k for k in range(Tj.bit_length()) if (Tj >> k) & 1]
                series_ops = [
                    lambda: V.tensor_scalar(SER[:], Se[:], c3s, -c2s, Alu.mult, Alu.add),
                    lambda: V.tensor_scalar(WSL[:], Se[:], Tf, 0.1, Alu.mult, Alu.is_lt),
                    lambda: V.tensor_tensor(SER[:], SER[:], Se[:], Alu.mult),
                    lambda: V.tensor_scalar(SER[:], SER[:], c1s, None, Alu.add),
                    lambda: V.tensor_tensor(PSS[:], SER[:], ataB, Alu.mult),
                    lambda: V.tensor_reduce(GD[:], TP[:], AX.X, Alu.add),
                ]
                si = 0
                PW = {0: Re}
                for k in range(1, bits[-1] + 1):
                    V.tensor_tensor(PWS[k][:], PW[k - 1][:], PW[k - 1][:], Alu.mult)
                    PW[k] = PWS[k]
                    if si < len(series_ops):
                        series_ops[si](); si += 1
                acc = PW[bits[-1]]
                for i, k in enumerate(reversed(bits[:-1])):
                    dst = EXe if i % 2 == 0 else LNe
                    V.tensor_tensor(dst[:], acc[:], PW[k][:], Alu.mult)
                    if si < len(series_ops):
                        series_ops[si](); si += 1
                    acc = dst
                while si < len(series_ops):
                    series_ops[si](); si += 1
                EXr = acc           # holds r^T
                # ---- psi -> theta ----
                V.tensor_scalar(NUMe[:], EXr[:], -1.0, 1.0, Alu.mult, Alu.add)
                V.tensor_tensor(PHI[:], NUMe[:], IED[:, 0:2, :], Alu.mult)
                V.tensor_tensor(PSI[:], atB, PHI[:], Alu.subtract)
                V.tensor_tensor(PSI[:], PSI[:], IED[:, 0:2, :], Alu.mult)
                # blend psi_series where s*T < 0.1
                V.tensor_tensor(PHI[:], PSS[:], PSI[:], Alu.subtract)
                V.tensor_tensor(PHI[:], PHI[:], WSL[:], Alu.mult)
                V.tensor_tensor(PSI[:], PSI[:], PHI[:], Alu.add)
                V.tensor_tensor(PSI[:], PSI[:], Linv2, Alu.mult)   # psi*Linv
                V.tensor_tensor(tA[:], PSI[:, 0], PSI[:, 1], Alu.subtract)
                V.tensor_tensor(TH1[:], tA[:], IED[:, 2, :], Alu.mult)
                V.tensor_tensor(tB[:], TH1[:], E3[:, 0, :], Alu.mult)
                V.tensor_tensor(F3[:], Sdg4[:, 0:3, :], th1B3, Alu.mult)
                V.tensor_tensor(TH0[:], PSI[:, 0], tB[:], Alu.subtract)
                V.tensor_tensor(F3[:, 0:2, :], F3[:, 0:2, :], th0B, Alu.add)
                V.tensor_tensor(B1t[:], GD[:], phoB, Alu.mult)     # [o*g1, o*g2]
                V.tensor_tensor(A1[:], F3[:, 0:2, :], GD[:], Alu.mult)
                V.tensor_tensor(Wv[:, 0], A1[:, 0], B1t[:, 1], Alu.add)
                V.tensor_tensor(Wv[:, 1], A1[:, 1], B1t[:, 0], Alu.add)
                # ---- apply: z' = relu(z) + T*delta - Gm w ----
                for i, hs in enumerate(halves):
                    V.tensor_tensor(TP[:, :, hs, :], GM[:, :, hs, :],
                                    Wv[:, :, hs].unsqueeze(3).broadcast_to(
                                        [P, 2, HB, M]), Alu.mult)
                for i, hs in enumerate(halves):
                    V.tensor_tensor(GS[:, hs, :], TP[:, 0, hs, :],
                                    TP[:, 1, hs, :], Alu.add)
                for i, hs in enumerate(halves):
                    V.scalar_tensor_tensor(GR[:, hs, :], Dlt[:, hs, :], Tf,
                                           GS[:, hs, :], Alu.mult, Alu.subtract)
                for i, hs in enumerate(halves):
                    V.scalar_tensor_tensor(lam[:, hs, :], lam[:, hs, :], 0.0,
                                           GR[:, hs, :], Alu.max, Alu.add)

            # ---------------- u = -p - G^T relu(lam) ----------------
            # Halved so the first half's (contiguous-layout) DMA issues
            # while the second half computes; the (b p) unscramble moved
            # to the host.
            u12 = scr.tile([P, BC, 2], f32)
            for i, hs in enumerate(halves):
                V.scalar_tensor_tensor(T[:, :, hs, :], lam_b2[i], 0.0,
                                       gxy[:, :, hs, :], Alu.max, Alu.mult)
            for i, hs in enumerate(halves):
                V.tensor_reduce(GL[:, :, hs], T[:, :, hs, :], AX.X, Alu.add)
            for i, hs in enumerate(halves):
                V.scalar_tensor_tensor(u12[:, hs, 0], GL[:, 0, hs], -1.0,
                                       pg[:, 0, hs], Alu.mult, Alu.subtract)
                V.scalar_tensor_tensor(u12[:, hs, 1], GL[:, 1, hs], -1.0,
                                       pg[:, 1, hs], Alu.mult, Alu.subtract)
                nc.sync.dma_start(y_d[:, hs, :], u12[:, hs, :])

    nc.finalize()
    _split_multi_waits(nc)
    return nc


_CACHED = {}


def _get_kernel():
    if "nc" not in _CACHED:
        _CACHED["nc"] = build_kernel()
    return _CACHED["nc"]


def _round_tf32(a):
    """RNE to 10-bit mantissa (TF32) so f32r consumers see pre-rounded data."""
    v = np.ascontiguousarray(np.asarray(a, np.float32)).view(np.uint32)
    r = v + np.uint32(0xFFF) + ((v >> np.uint32(13)) & np.uint32(1))
    r &= np.uint32(0xFFFFE000)
    return r.view(np.float32)


def build_in_maps(inputs):
    x = np.ascontiguousarray(np.asarray(inputs["x"], dtype=np.float32))
    obstacles = np.asarray(inputs["obstacles"], dtype=np.float32)
    std = np.asarray(inputs["std"], dtype=np.float32)
    mean = np.asarray(inputs["mean"], dtype=np.float32)

    rw = _round_tf32
    shared = {
        "W1": rw(inputs["W1"]),
        "b1": np.ascontiguousarray(np.asarray(inputs["b1"], np.float32)),
        "W21": rw(inputs["W21"]),
        "b21": np.ascontiguousarray(np.asarray(inputs["b21"], np.float32)),
        "W22": rw(inputs["W22"]),
        "b22": np.ascontiguousarray(np.asarray(inputs["b22"], np.float32)),
        "W31": rw(inputs["W31"]),
        "b31": np.ascontiguousarray(np.asarray(inputs["b31"], np.float32)),
        "W32": rw(inputs["W32"]),
        "b32": np.ascontiguousarray(np.asarray(inputs["b32"], np.float32)),
        "id4": np.eye(4, dtype=np.float32),
        "obsb": np.ascontiguousarray(
            np.broadcast_to(obstacles.T[None, :, :], (P, 3, 8)).astype(np.float32)),
        "stdb": np.ascontiguousarray(np.broadcast_to(std[None, :], (P, 8))),
        "meanb": np.ascontiguousarray(np.broadcast_to(mean[None, :], (P, 8))),
    }

    in_maps = []
    for c in range(N_CORES):
        xe = x[c * B_CORE:(c + 1) * B_CORE]            # [1024, 8]
        m = dict(shared)
        m["xT"] = rw(xe.T)                             # [8, 1024] (TF32-rounded)
        # sample-layout gather for the barrier math:
        m["xsg"] = np.ascontiguousarray(
            xe.reshape(BC, P, 8).transpose(1, 2, 0))   # [p, f, b]
        in_maps.append(m)
    return in_maps


def kernel(**inputs):
    in_maps = build_in_maps(inputs)
    nc = _get_kernel()
    res = run_bass_kernel_spmd(nc, in_maps, core_ids=list(range(N_CORES)))
    out = np.concatenate(
        [np.asarray(res.results[c]["y"]).transpose(1, 0, 2).reshape(B_CORE, 2)
         for c in range(N_CORES)], axis=0)
    return out.astype(np.float32)
